# revision 7
# baseline (speedup 1.0000x reference)
"""Trainium2 Bass kernel for a BasicTransformerBlock (self-attn + cross-attn +
GEGLU FF, all with rank-16 LoRA deltas), sharded over 8 NeuronCores.

Sharding: core c handles batch b = c//4 and a 512-token quarter of the 2048
token sequence. Each core's x^T input is rotated so its own tokens are columns
0:512; self-attention K/V are computed for the full (rotated) sequence, which
is valid because softmax attention is permutation-invariant over keys.

Layout: activations are kept transposed (model dim on partitions, tokens on
the free axis). LayerNorm reductions over the model dim use ones-vector
matmuls; softmax denominators come from a ones-column appended to V; per-token
broadcasts use K=1 matmuls.

Dtypes: float32r (fp32 storage, reduced-precision PE multiply, full-rate) for
projections/FF/LN; bf16 for attention internals (Q/K/V/exp(S)/O and the
attention output projection) to fit SBUF.
"""

import sys

sys.path.insert(0, "/opt/trn_rl_repo")

import numpy as np
import ml_dtypes

import concourse.bacc as bacc
import concourse.tile as tile
from concourse import mybir
from concourse.bass_utils import run_bass_kernel_spmd

F32R = mybir.dt.float32r
F32 = mybir.dt.float32
BF16 = mybir.dt.bfloat16
AF = mybir.ActivationFunctionType
ALU = mybir.AluOpType
BF16NP = ml_dtypes.bfloat16

DIM, CTX_DIM, HEADS, DHEAD = 1024, 768, 16, 64
NCH, CCH = DIM // 128, CTX_DIM // 128          # 8, 6
TO, TF, NTT = 512, 2048, 4                     # own tokens, full tokens, tiles
NCTX = 77
NCTXP = 80                                     # padded for f32r even-N rule
F2, F1 = 8192, 4096                            # GEGLU proj, inner
RANK = 16
N_CORES = 8


def _build(nc):
    from contextlib import ExitStack

    dram = {}

    def din(name, shape, dt=F32R):
        dram[name] = nc.dram_tensor(name, shape, dt, kind="ExternalInput")
        return dram[name]

    xT = din("xT", [DIM, TF])
    ctxT = din("ctxT", [CTX_DIM, NCTXP])
    consts = din("consts", [128, 129])            # cols 0:128 = 1, col 128 = 1/1024
    constsb = din("constsb", [128, 256], BF16)    # all ones
    w_q = din("w_q", [DIM, DIM]); w_k = din("w_k", [DIM, DIM])
    w_v = din("w_v", [DIM, DIM]); w_o = din("w_o", [DIM, DIM], BF16)
    din("b_o", [DIM], F32)
    d_qkv = din("d_qkv", [DIM, 96])               # q@0:16, k@32:48, v@64:80
    u_q = din("u_q", [RANK, DIM]); u_k = din("u_k", [RANK, DIM])
    u_v = din("u_v", [RANK, DIM])
    d_o = din("d_o", [DIM, RANK], BF16); u_o = din("u_o", [RANK, DIM], BF16)
    w_q2 = din("w_q2", [DIM, DIM])
    w_k2 = din("w_k2", [CTX_DIM, DIM]); w_v2 = din("w_v2", [CTX_DIM, DIM])
    w_o2 = din("w_o2", [DIM, DIM], BF16); din("b_o2", [DIM], F32)
    d_q2 = din("d_q2", [DIM, RANK]); u_q2 = din("u_q2", [RANK, DIM])
    d_kv2 = din("d_kv2", [CTX_DIM, 48])           # k2@0:16, v2@32:48
    u_k2 = din("u_k2", [RANK, DIM]); u_v2 = din("u_v2", [RANK, DIM])
    d_o2 = din("d_o2", [DIM, RANK], BF16); u_o2 = din("u_o2", [RANK, DIM], BF16)
    for nm in ("ln1g", "ln1b", "ln2g", "ln2b", "ln3g", "ln3b"):
        din(nm, [DIM], F32)
    w_p = din("w_p", [DIM, F2]); b_p = din("b_p", [F2], F32)
    d_p = din("d_p", [DIM, RANK]); u_p = din("u_p", [RANK, F2])
    w_2 = din("w_2", [F1, DIM]); din("b_2", [DIM], F32)
    d_f = din("d_f", [F1, RANK]); u_f = din("u_f", [RANK, DIM])
    outT = nc.dram_tensor("outT", [DIM, TO], F32, kind="ExternalOutput")

    def chunked(t):
        # [C*128, N] dram -> [128, C, N] access pattern
        return t.rearrange("(c p) n -> p c n", p=128)

    with tile.TileContext(nc) as tc, \
            nc.allow_low_precision(reason="f32r/bf16 kernel by design"), \
            ExitStack() as ctx:
        cst = ctx.enter_context(tc.tile_pool(name="cst", bufs=1))
        sb = ctx.enter_context(tc.tile_pool(name="sb", bufs=1))

        # ---------------- constants ----------------
        o_col = cst.tile([128, 1], F32R, tag="o_col")       # 1/1024
        nc.sync.dma_start(out=o_col, in_=consts[:, 128:129])
        o_row = cst.tile([1, 128], F32R, tag="o_row")       # ones row (bcast lhsT)
        nc.sync.dma_start(out=o_row, in_=consts[0:1, 0:128])
        ones64 = cst.tile([65, 64], F32R, tag="ones64")     # row 64 = ones
        nc.sync.dma_start(out=ones64[64:65, :], in_=consts[0:1, 0:64])
        eps_t = cst.tile([1, 1], F32, tag="eps")
        nc.vector.memset(eps_t, 1e-5)

        lnp = {}
        for nm in ("ln1g", "ln1b", "ln2g", "ln2b", "ln3g", "ln3b",
                   "b_o", "b_o2", "b_2"):
            t = cst.tile([128, NCH], F32, tag=nm)
            nc.sync.dma_start(out=t, in_=dram[nm].rearrange("(c p) -> p c", p=128))
            lnp[nm] = t
        bp_t = cst.tile([128, F2 // 128], F32, tag="bp")
        nc.sync.dma_start(out=bp_t, in_=b_p.rearrange("(c p) -> p c", p=128))

        d_qkv_t = cst.tile([128, NCH, 96], F32R, tag="d_qkv")
        nc.sync.dma_start(out=d_qkv_t, in_=chunked(d_qkv))
        d_o_t = cst.tile([128, NCH, RANK], BF16, tag="d_o")
        nc.sync.dma_start(out=d_o_t, in_=chunked(d_o))
        d_q2_t = cst.tile([128, NCH, RANK], F32R, tag="d_q2")
        nc.sync.dma_start(out=d_q2_t, in_=chunked(d_q2))
        d_kv2_t = cst.tile([128, CCH, 48], F32R, tag="d_kv2")
        nc.sync.dma_start(out=d_kv2_t, in_=chunked(d_kv2))
        d_o2_t = cst.tile([128, NCH, RANK], BF16, tag="d_o2")
        nc.sync.dma_start(out=d_o2_t, in_=chunked(d_o2))
        d_p_t = cst.tile([128, NCH, RANK], F32R, tag="d_p")
        nc.sync.dma_start(out=d_p_t, in_=chunked(d_p))
        d_f_t = cst.tile([128, F1 // 128, RANK], F32R, tag="d_f")
        nc.sync.dma_start(out=d_f_t, in_=chunked(d_f))

        # ---------------- big persistent tiles (tag-recycled) ----------------
        K_all = sb.tile([128, NCH, TF], BF16, tag="kx")      # K^T, later h_lo
        V_all = sb.tile([128, 16, 16 * 65], BF16, tag="vx")  # V (65-padded), later h_hi
        Q_all = sb.tile([128, NCH, TO], BF16, tag="qx")      # Q^T, later Q2
        O_all = sb.tile([128, NCH, TO], BF16, tag="ox")      # O^T, later O2

        # ones columns of V (col 64 of every 65-block)
        nc.sync.dma_start(
            out=V_all.rearrange("p n (h e) -> p (n h) e", e=65)[:, :, 64:65],
            in_=constsb[:, 0:256].rearrange("p (a b) -> p a b", b=1))

        # ---------------- helpers ----------------
        def ln_stats(ps_pool, src, n_src):
            """mean/rstd over partitions via ones-matmuls. src[c]: [128, T] f32r."""
            T = src[0].shape[-1]
            m_ps = ps_pool.tile([1, T], F32, tag="st", bufs=2)
            m2_ps = ps_pool.tile([1, T], F32, tag="st", bufs=2)
            for c in range(n_src):
                xsq = sb.tile([128, T], F32R, tag="xsq", bufs=2)
                nc.scalar.activation(out=xsq, in_=src[c], func=AF.Square)
                nc.tensor.matmul(m_ps, o_col, src[c], start=(c == 0),
                                 stop=(c == n_src - 1))
                nc.tensor.matmul(m2_ps, o_col, xsq, start=(c == 0),
                                 stop=(c == n_src - 1))
            m_sb = sb.tile([1, T], F32R, tag="sst", bufs=4)
            nc.scalar.activation(out=m_sb, in_=m_ps, func=AF.Copy)
            msq = sb.tile([1, T], F32, tag="sst", bufs=4)
            nc.vector.tensor_mul(out=msq, in0=m_sb, in1=m_sb)
            var = sb.tile([1, T], F32, tag="sst", bufs=4)
            nc.vector.tensor_sub(out=var, in0=m2_ps, in1=msq)
            nc.scalar.activation(out=var, in_=var, func=AF.Sqrt, bias=eps_t)
            rstd = sb.tile([1, T], F32R, tag="sst", bufs=4)
            nc.vector.reciprocal(out=rstd, in_=var)
            return m_sb, rstd

        def ln_apply(ps_pool, src, dst, m_sb, rstd, gkey, bkey, n_src):
            T = src[0].shape[-1]
            m_bc = ps_pool.tile([128, T], F32, tag="bc", bufs=2)
            nc.tensor.matmul(m_bc, o_row, m_sb, start=True, stop=True)
            r_bc = ps_pool.tile([128, T], F32, tag="bc", bufs=2)
            nc.tensor.matmul(r_bc, o_row, rstd, start=True, stop=True)
            g_t, b_t = lnp[gkey], lnp[bkey]
            for c in range(n_src):
                nc.vector.tensor_sub(out=dst[c], in0=src[c], in1=m_bc)
                nc.vector.tensor_mul(out=dst[c], in0=dst[c], in1=r_bc)
                nc.vector.tensor_scalar(out=dst[c], in0=dst[c],
                                        scalar1=g_t[:, c:c + 1],
                                        scalar2=b_t[:, c:c + 1],
                                        op0=ALU.mult, op1=ALU.add)

        def u_slice(u_dram, lo, hi, dt=F32R, row0=0):
            t = sb.tile([row0 + RANK, hi - lo], dt, tag="ust", bufs=2,
                        name="ust")
            nc.sync.dma_start(out=t[row0:row0 + RANK, :], in_=u_dram[:, lo:hi])
            return t[row0:row0 + RANK, :]

        def proj_T(ps_pool, w_dram, src, out_write, lora, n_c=NCH, dt=F32R,
                   n_free=TO, lrow=0):
            """out^T[ic] = sum_c W[c,ic].T @ src[c] + lora up. out_write(ic, ps)."""
            u_dram, dn_rhs, ldt = lora
            for h in range(2):
                pss = [ps_pool.tile([128, n_free], F32, tag="pj", bufs=4,
                                    name=f"pj{h}_{i}") for i in range(4)]
                for c in range(n_c):
                    wt = sb.tile([128, 512], dt, tag="wst", bufs=2)
                    nc.sync.dma_start(out=wt,
                                      in_=chunked(w_dram)[:, c, h * 512:(h + 1) * 512])
                    for i in range(4):
                        nc.tensor.matmul(pss[i], wt[:, i * 128:(i + 1) * 128],
                                         src[c], start=(c == 0), stop=False)
                for i in range(4):
                    ic = 4 * h + i
                    ut = u_slice(u_dram, ic * 128, (ic + 1) * 128, ldt,
                                 row0=lrow)
                    nc.tensor.matmul(pss[i], ut, dn_rhs, start=False, stop=True)
                    out_write(ic, pss[i])

        def attn(ps_pool, q_all, k_all, v_all, n_k_chunks, n_k, o_all):
            """q_all [128, NCH, TO]; k_all [128, NCH, n_k]; v_all(tc) -> V tile."""
            for hd in range(HEADS):
                ic, p0 = hd // 2, 64 * (hd % 2)
                av = ps_pool.tile([65, TO], F32, tag="av", bufs=2)
                for tcc in range(n_k_chunks):
                    k_lo = tcc * 128
                    k_n = min(128, n_k - k_lo)
                    s_ps = ps_pool.tile([k_n, TO], F32, tag="sx", bufs=3)
                    nc.tensor.matmul(
                        s_ps, k_all[p0:p0 + 64, ic, k_lo:k_lo + k_n],
                        q_all[p0:p0 + 64, ic, :], start=True, stop=True)
                    es = sb.tile([k_n, TO], BF16, tag="es", bufs=2)
                    nc.scalar.activation(out=es, in_=s_ps, func=AF.Exp,
                                         scale=float(DHEAD) ** -0.5)
                    nc.tensor.matmul(av, v_all(tcc)[0:k_n, hd * 65:(hd + 1) * 65],
                                     es, start=(tcc == 0),
                                     stop=(tcc == n_k_chunks - 1))
                den = sb.tile([65, TO], F32R, tag="den", bufs=1)
                nc.scalar.activation(out=den[64:65, :], in_=av[64:65, :],
                                     func=AF.Copy)
                nc.vector.reciprocal(out=den[64:65, :], in_=den[64:65, :])
                rd = ps_pool.tile([64, TO], F32, tag="rd", bufs=2)
                nc.tensor.matmul(rd, ones64[64:65, :], den[64:65, :],
                                 start=True, stop=True)
                o_cp = sb.tile([64, TO], F32, tag="cp", bufs=3)
                nc.scalar.activation(out=o_cp, in_=av[0:64, :], func=AF.Copy)
                if p0 == 0:
                    nc.vector.tensor_mul(out=o_all[0:64, ic, :], in0=o_cp, in1=rd)
                else:
                    o_tmp = sb.tile([64, TO], BF16, tag="ot", bufs=1)
                    nc.vector.tensor_mul(out=o_tmp, in0=o_cp, in1=rd)
                    nc.sync.dma_start(out=o_all[p0:p0 + 64, ic, :], in_=o_tmp)

        def wo_phase(ps_pool, w_dram, o_all, d_t, u_dram, bias_key, x_res, x2_dst):
            """x2 = x_res + W_o.T@O + u_o.T@(d_o.T@O) + b_o   (bf16 weights)."""
            od_ps = ps_pool.tile([RANK, TO], F32, tag="pj", bufs=4)
            for c in range(NCH):
                nc.tensor.matmul(od_ps, d_t[:, c, :], o_all[:, c, :],
                                 start=(c == 0), stop=(c == NCH - 1))
            od_sb = sb.tile([RANK, TO], BF16, tag="odx", bufs=2)
            nc.scalar.activation(out=od_sb, in_=od_ps, func=AF.Copy)
            for h in range(2):
                pss = [ps_pool.tile([128, TO], F32, tag="pj", bufs=4,
                                    name=f"pjo{h}_{i}") for i in range(4)]
                for c in range(NCH):
                    wt = sb.tile([128, 512], BF16, tag="wst", bufs=2)
                    nc.sync.dma_start(out=wt,
                                      in_=chunked(w_dram)[:, c, h * 512:(h + 1) * 512])
                    for i in range(4):
                        nc.tensor.matmul(pss[i], wt[:, i * 128:(i + 1) * 128],
                                         o_all[:, c, :], start=(c == 0), stop=False)
                for i in range(4):
                    dc = 4 * h + i
                    ut = u_slice(u_dram, dc * 128, (dc + 1) * 128, BF16)
                    nc.tensor.matmul(pss[i], ut, od_sb, start=False, stop=True)
                    t = sb.tile([128, TO], F32, tag="cp", bufs=3)
                    nc.scalar.activation(out=t, in_=pss[i], func=AF.Identity,
                                         bias=lnp[bias_key][:, dc:dc + 1])
                    nc.vector.tensor_add(out=x2_dst[:, dc, :], in0=t,
                                         in1=x_res(dc))

        # ======================= phase A: LN1 + Q/K/V =======================

        with tc.tile_pool(name="psA", bufs=1, space="PSUM") as psA:
            for tt in range(NTT):
                x_tt = sb.tile([128, NCH, TO], F32R, tag="xs", bufs=1)
                nc.sync.dma_start(out=x_tt,
                                  in_=chunked(xT)[:, :, tt * TO:(tt + 1) * TO])
                xs = [x_tt[:, c, :] for c in range(NCH)]
                m_sb, rstd = ln_stats(psA, xs, NCH)
                h1 = sb.tile([128, NCH, TO], F32R, tag="h1", bufs=1)
                h1c = [h1[:, c, :] for c in range(NCH)]
                ln_apply(psA, xs, h1c, m_sb, rstd, "ln1g", "ln1b", NCH)

                # packed qkv lora down: [96, TO]
                xd_ps = psA.tile([96, TO], F32, tag="pj", bufs=4)
                for c in range(NCH):
                    nc.tensor.matmul(xd_ps, d_qkv_t[:, c, :], h1c[c],
                                     start=(c == 0), stop=(c == NCH - 1))
                xd_tt = sb.tile([96, TO], F32R, tag="xd", bufs=2)
                nc.scalar.activation(out=xd_tt, in_=xd_ps, func=AF.Copy)

                if tt == 0:
                    def wq_out(ic, ps):
                        nc.scalar.activation(out=Q_all[:, ic, :], in_=ps,
                                             func=AF.Copy)
                    proj_T(psA, w_q, h1c, wq_out,
                           lora=(u_q, xd_tt[0:16, :], F32R))

                def wk_out(ic, ps, _tt=tt):
                    nc.scalar.activation(
                        out=K_all[:, ic, _tt * TO:(_tt + 1) * TO], in_=ps,
                        func=AF.Copy)
                proj_T(psA, w_k, h1c, wk_out,
                       lora=(u_k, xd_tt[32:48, :], F32R), lrow=32)

                # V natural layout, 65-padded heads
                for half in range(2):
                    ps_v = [psA.tile([128, TO], F32, tag="pj", bufs=4,
                                     name=f"psv{half}_{i}") for i in range(4)]
                    for c in range(NCH):
                        wt = sb.tile([128, 512], F32R, tag="wst", bufs=2)
                        nc.sync.dma_start(
                            out=wt,
                            in_=chunked(w_v)[:, c, half * 512:(half + 1) * 512])
                        for tcc in range(4):
                            nc.tensor.matmul(
                                ps_v[tcc], h1[:, c, tcc * 128:(tcc + 1) * 128],
                                wt, start=(c == 0), stop=False)
                    ut = u_slice(u_v, half * 512, (half + 1) * 512, row0=64)
                    for tcc in range(4):
                        nc.tensor.matmul(
                            ps_v[tcc],
                            xd_tt[64:80, tcc * 128:(tcc + 1) * 128],
                            ut, start=False, stop=True)
                        vtile = V_all[:, tt * 4 + tcc, :] \
                            .rearrange("p (h e) -> p h e", e=65)
                        nc.scalar.activation(
                            out=vtile[:, half * 8:(half + 1) * 8, 0:64],
                            in_=ps_v[tcc].rearrange("p (h e) -> p h e", e=64),
                            func=AF.Copy)

        # ======================= phase B: self-attention ====================
        with tc.tile_pool(name="psB", bufs=1, space="PSUM") as psB:
            attn(psB, Q_all, K_all, lambda tcc: V_all[:, tcc, :], TF // 128, TF,
                 O_all)

        # ======================= phase C: Wo + residual =====================
        x2_all = sb.tile([128, NCH, TO], F32R, tag="x2")
        with tc.tile_pool(name="psC", bufs=1, space="PSUM") as psC:
            xres = sb.tile([128, NCH, TO], F32R, tag="xs", bufs=1)
            nc.sync.dma_start(out=xres, in_=chunked(xT)[:, :, 0:TO])
            wo_phase(psC, w_o, O_all, d_o_t, u_o, "b_o",
                     lambda dc: xres[:, dc, :], x2_all)

        # ============== phase D: LN2, cross-attention =======================
        Q2_all = sb.tile([128, NCH, TO], BF16, tag="qx")
        K2_all = sb.tile([128, NCH, NCTXP], BF16, tag="k2")
        V2_t = sb.tile([128, 16 * 65], BF16, tag="v2")
        nc.sync.dma_start(
            out=V2_t.rearrange("p (h e) -> p h e", e=65)[:, :, 64:65],
            in_=constsb[:, 0:16].rearrange("p (a b) -> p a b", b=1))
        ctx_t = sb.tile([128, CCH, NCTXP], F32R, tag="ctx")
        nc.sync.dma_start(out=ctx_t, in_=chunked(ctxT))
        O2_all = sb.tile([128, NCH, TO], BF16, tag="ox")

        with tc.tile_pool(name="psD", bufs=1, space="PSUM") as psD:
            x2c = [x2_all[:, c, :] for c in range(NCH)]
            m_sb, rstd = ln_stats(psD, x2c, NCH)
            h2 = sb.tile([128, NCH, TO], F32R, tag="h1", bufs=1)
            h2c = [h2[:, c, :] for c in range(NCH)]
            ln_apply(psD, x2c, h2c, m_sb, rstd, "ln2g", "ln2b", NCH)

            q2d_ps = psD.tile([RANK, TO], F32, tag="pj", bufs=4)
            for c in range(NCH):
                nc.tensor.matmul(q2d_ps, d_q2_t[:, c, :], h2c[c],
                                 start=(c == 0), stop=(c == NCH - 1))
            q2d_sb = sb.tile([RANK, TO], F32R, tag="odx", bufs=2)
            nc.scalar.activation(out=q2d_sb, in_=q2d_ps, func=AF.Copy)

            def wq2_out(ic, ps):
                nc.scalar.activation(out=Q2_all[:, ic, :], in_=ps, func=AF.Copy)
            proj_T(psD, w_q2, h2c, wq2_out, lora=(u_q2, q2d_sb, F32R))

            # kv2 lora down from raw context
            cd_ps = psD.tile([48, NCTXP], F32, tag="pj", bufs=4)
            for c in range(CCH):
                nc.tensor.matmul(cd_ps, d_kv2_t[:, c, :], ctx_t[:, c, :],
                                 start=(c == 0), stop=(c == CCH - 1))
            cd_sb = sb.tile([48, NCTXP], F32R, tag="odx", bufs=2)
            nc.scalar.activation(out=cd_sb, in_=cd_ps, func=AF.Copy)

            # K2^T
            for h in range(2):
                pss = [psD.tile([128, NCTXP], F32, tag="pj", bufs=4,
                                name=f"pk2{h}_{i}") for i in range(4)]
                for c in range(CCH):
                    wt = sb.tile([128, 512], F32R, tag="wst", bufs=2)
                    nc.sync.dma_start(
                        out=wt, in_=chunked(w_k2)[:, c, h * 512:(h + 1) * 512])
                    for i in range(4):
                        nc.tensor.matmul(pss[i], wt[:, i * 128:(i + 1) * 128],
                                         ctx_t[:, c, :], start=(c == 0), stop=False)
                for i in range(4):
                    ic = 4 * h + i
                    ut = u_slice(u_k2, ic * 128, (ic + 1) * 128)
                    nc.tensor.matmul(pss[i], ut, cd_sb[0:16, :], start=False,
                                     stop=True)
                    nc.scalar.activation(out=K2_all[:, ic, :], in_=pss[i],
                                         func=AF.Copy)
            # V2 natural
            for half in range(2):
                ps_v = psD.tile([NCTX, 512], F32, tag="pj", bufs=4)
                for c in range(CCH):
                    wt = sb.tile([128, 512], F32R, tag="wst", bufs=2)
                    nc.sync.dma_start(
                        out=wt, in_=chunked(w_v2)[:, c, half * 512:(half + 1) * 512])
                    nc.tensor.matmul(ps_v, ctx_t[:, c, 0:NCTX], wt,
                                     start=(c == 0), stop=False)
                ut = u_slice(u_v2, half * 512, (half + 1) * 512, row0=32)
                nc.tensor.matmul(ps_v, cd_sb[32:48, 0:NCTX], ut, start=False, stop=True)
                nc.scalar.activation(
                    out=V2_t.rearrange("p (h e) -> p h e", e=65)
                        [0:NCTX, half * 8:(half + 1) * 8, 0:64],
                    in_=ps_v.rearrange("p (h e) -> p h e", e=64),
                    func=AF.Copy)

        with tc.tile_pool(name="psD2", bufs=1, space="PSUM") as psD2:
            attn(psD2, Q2_all, K2_all, lambda tcc: V2_t, 1, NCTX, O2_all)

        # ======================= phase E: Wo2 + residual ====================
        x3_all = sb.tile([128, NCH, TO], F32R, tag="x3")
        with tc.tile_pool(name="psE", bufs=1, space="PSUM") as psE:
            wo_phase(psE, w_o2, O2_all, d_o2_t, u_o2, "b_o2",
                     lambda dc: x2_all[:, dc, :], x3_all)

        # ======================= phase F: LN3 ===============================
        h3 = sb.tile([128, NCH, TO], F32R, tag="h1", bufs=1)
        with tc.tile_pool(name="psF", bufs=1, space="PSUM") as psF:
            x3c = [x3_all[:, c, :] for c in range(NCH)]
            m_sb, rstd = ln_stats(psF, x3c, NCH)
            h3c = [h3[:, c, :] for c in range(NCH)]
            ln_apply(psF, x3c, h3c, m_sb, rstd, "ln3g", "ln3b", NCH)

        # ======================= phase G: GEGLU FF ==========================
        h_lo = sb.tile([128, 16, TO], F32R, tag="kx")   # h chunks 0..15
        h_hi = sb.tile([128, 16, TO], F32R, tag="vx")   # h chunks 16..31
        with tc.tile_pool(name="psG", bufs=1, space="PSUM") as psG:
            h3c = [h3[:, c, :] for c in range(NCH)]
            pd_ps = psG.tile([RANK, TO], F32, tag="pj", bufs=4)
            for c in range(NCH):
                nc.tensor.matmul(pd_ps, d_p_t[:, c, :], h3c[c],
                                 start=(c == 0), stop=(c == NCH - 1))
            pd_sb = sb.tile([RANK, TO], F32R, tag="odx", bufs=2)
            nc.scalar.activation(out=pd_sb, in_=pd_ps, func=AF.Copy)

            def proj_block(j, gate):
                """proj^T chunks for col block j (4 chunks of 128).
                Returns [(fc, psum_tile)]."""
                base = (32 if gate else 0) + 4 * j
                pss = [psG.tile([128, TO], F32, tag="pj", bufs=4,
                                name=f"pg{h}_{i}") for i in range(4)]
                for c in range(NCH):
                    wt = sb.tile([128, 512], F32R, tag="wst", bufs=2)
                    nc.sync.dma_start(
                        out=wt,
                        in_=chunked(w_p)[:, c, base * 128:(base + 4) * 128])
                    for i in range(4):
                        nc.tensor.matmul(pss[i], wt[:, i * 128:(i + 1) * 128],
                                         h3c[c], start=(c == 0), stop=False)
                out = []
                for i in range(4):
                    fc = base + i
                    ut = u_slice(u_p, fc * 128, (fc + 1) * 128)
                    nc.tensor.matmul(pss[i], ut, pd_sb, start=False, stop=True)
                    out.append((fc, pss[i]))
                return out

            def h_ap(fc):
                return h_lo[:, fc, :] if fc < 16 else h_hi[:, fc - 16, :]

            for j in range(8):
                gels = []
                for fc, ps in proj_block(j, gate=True):
                    gel = sb.tile([128, TO], F32, tag="gel", bufs=4)
                    nc.scalar.activation(out=gel, in_=ps, func=AF.Gelu,
                                         bias=bp_t[:, fc:fc + 1])
                    gels.append(gel)
                for idx, (fc, ps) in enumerate(proj_block(j, gate=False)):
                    t = sb.tile([128, TO], F32, tag="cp", bufs=3)
                    nc.vector.tensor_scalar_add(out=t, in0=ps,
                                                scalar1=bp_t[:, fc:fc + 1])
                    nc.vector.tensor_mul(out=h_ap(fc), in0=t, in1=gels[idx])

            # second FF matmul + bias + residual -> outT
            hd_ps = psG.tile([RANK, TO], F32, tag="pj", bufs=4)
            for fc in range(32):
                nc.tensor.matmul(hd_ps, d_f_t[:, fc, :], h_ap(fc),
                                 start=(fc == 0), stop=(fc == 31))
            hd_sb = sb.tile([RANK, TO], F32R, tag="odx", bufs=2)
            nc.scalar.activation(out=hd_sb, in_=hd_ps, func=AF.Copy)
            for h in range(2):
                pss = [psG.tile([128, TO], F32, tag="pj", bufs=4,
                                name=f"pg{h}_{i}") for i in range(4)]
                for fc in range(32):
                    wt = sb.tile([128, 512], F32R, tag="wst", bufs=2)
                    nc.sync.dma_start(
                        out=wt, in_=chunked(w_2)[:, fc, h * 512:(h + 1) * 512])
                    for i in range(4):
                        nc.tensor.matmul(pss[i], wt[:, i * 128:(i + 1) * 128],
                                         h_ap(fc), start=(fc == 0), stop=False)
                for i in range(4):
                    dc = 4 * h + i
                    ut = u_slice(u_f, dc * 128, (dc + 1) * 128)
                    nc.tensor.matmul(pss[i], ut, hd_sb, start=False, stop=True)
                    t = sb.tile([128, TO], F32, tag="cp", bufs=3)
                    nc.scalar.activation(out=t, in_=pss[i], func=AF.Identity,
                                         bias=lnp["b_2"][:, dc:dc + 1])
                    of = sb.tile([128, TO], F32, tag="gel", bufs=4)
                    nc.vector.tensor_add(out=of, in0=t, in1=x3_all[:, dc, :])
                    nc.sync.dma_start(out=outT[dc * 128:(dc + 1) * 128, :], in_=of)

    nc.finalize()
    return nc


_CACHE = {}


def _get_nc():
    if "nc" not in _CACHE:
        _CACHE["nc"] = _build(bacc.Bacc())
    return _CACHE["nc"]


def _prep_in_maps(x, context, params):
    p = params
    sc = lambda a: float(a) * 1.0 / RANK  # LORA_W * alpha / rank

    def f32(a):
        return np.ascontiguousarray(np.asarray(a, dtype=np.float32))

    def bf(a):
        return np.ascontiguousarray(np.asarray(a, dtype=np.float32)
                                    .astype(BF16NP))

    a1, a2, ff = p["attn1"], p["attn2"], p["ff"]
    shared = {
        "consts": np.concatenate(
            [np.ones((128, 128), np.float32),
             np.full((128, 1), 1.0 / DIM, np.float32)], 1),
        "constsb": np.ones((128, 256), np.float32).astype(BF16NP),
        "w_q": f32(a1["Wq"]), "w_k": f32(a1["Wk"]), "w_v": f32(a1["Wv"]),
        "w_o": bf(a1["Wo"]), "b_o": f32(a1["bo"]),
        "u_q": f32(np.asarray(a1["qu"]) * sc(a1["qa"])),
        "u_k": f32(np.asarray(a1["ku"]) * sc(a1["ka"])),
        "u_v": f32(np.asarray(a1["vu"]) * sc(a1["va"])),
        "d_o": bf(a1["od"]), "u_o": bf(np.asarray(a1["ou"]) * sc(a1["oa"])),
        "w_q2": f32(a2["Wq"]), "w_k2": f32(a2["Wk"]), "w_v2": f32(a2["Wv"]),
        "w_o2": bf(a2["Wo"]), "b_o2": f32(a2["bo"]),
        "d_q2": f32(a2["qd"]), "u_q2": f32(np.asarray(a2["qu"]) * sc(a2["qa"])),
        "u_k2": f32(np.asarray(a2["ku"]) * sc(a2["ka"])),
        "u_v2": f32(np.asarray(a2["vu"]) * sc(a2["va"])),
        "d_o2": bf(a2["od"]), "u_o2": bf(np.asarray(a2["ou"]) * sc(a2["oa"])),
        "ln1g": f32(p["ln1_g"]), "ln1b": f32(p["ln1_b"]),
        "ln2g": f32(p["ln2_g"]), "ln2b": f32(p["ln2_b"]),
        "ln3g": f32(p["ln3_g"]), "ln3b": f32(p["ln3_b"]),
        "w_p": f32(ff["Wp"]), "b_p": f32(ff["bp"]),
        "d_p": f32(ff["pd"]), "u_p": f32(np.asarray(ff["pu"]) * sc(ff["pa"])),
        "w_2": f32(ff["W2"]), "b_2": f32(ff["b2"]),
        "d_f": f32(ff["fd"]), "u_f": f32(np.asarray(ff["fu"]) * sc(ff["fa"])),
    }
    d_qkv = np.zeros((DIM, 96), np.float32)
    d_qkv[:, 0:16] = np.asarray(a1["qd"]); d_qkv[:, 32:48] = np.asarray(a1["kd"])
    d_qkv[:, 64:80] = np.asarray(a1["vd"])
    shared["d_qkv"] = d_qkv
    d_kv2 = np.zeros((CTX_DIM, 48), np.float32)
    d_kv2[:, 0:16] = np.asarray(a2["kd"]); d_kv2[:, 32:48] = np.asarray(a2["vd"])
    shared["d_kv2"] = d_kv2

    x = np.asarray(x, np.float32)
    context = np.asarray(context, np.float32)
    in_maps = []
    for core in range(N_CORES):
        b, t0 = core // 4, (core % 4) * TO
        xt = x[b].T  # [DIM, TF]
        m = dict(shared)
        m["xT"] = np.ascontiguousarray(
            np.concatenate([xt[:, t0:], xt[:, :t0]], axis=1))
        ctp = np.zeros((CTX_DIM, NCTXP), np.float32)
        ctp[:, :NCTX] = context[b].T
        m["ctxT"] = ctp
        in_maps.append(m)
    return in_maps


def run_spmd(in_maps, **kw):
    return run_bass_kernel_spmd(_get_nc(), in_maps,
                                core_ids=list(range(N_CORES)), **kw)


def kernel(x, context, params):
    in_maps = _prep_in_maps(x, context, params)
    res = run_spmd(in_maps)
    B, N = np.asarray(x).shape[:2]
    out = np.empty((B, N, DIM), np.float32)
    for core in range(N_CORES):
        b, t0 = core // 4, (core % 4) * TO
        out[b, t0:t0 + TO, :] = res.results[core]["outT"].T
    return out


# revision 8
# speedup vs baseline: 1.1323x; 1.1323x over previous
"""Trainium2 Bass kernel for a BasicTransformerBlock (self-attn + cross-attn +
GEGLU FF, all with rank-16 LoRA deltas), sharded over 8 NeuronCores.

Sharding: core c handles batch b = c//4 and a 512-token quarter of the 2048
token sequence. Each core's x^T input is rotated so its own tokens are columns
0:512; self-attention K/V are computed for the full (rotated) sequence, which
is valid because softmax attention is permutation-invariant over keys.

Layout: activations are kept transposed (model dim on partitions, tokens on
the free axis). LayerNorm reductions over the model dim use ones-vector
matmuls; softmax denominators come from a ones-column appended to V; per-token
broadcasts use K=1 matmuls.

Dtypes: float32r (fp32 storage, reduced-precision PE multiply, full-rate) for
projections/FF/LN; bf16 for attention internals (Q/K/V/exp(S)/O and the
attention output projection) to fit SBUF.
"""

import sys

sys.path.insert(0, "/opt/trn_rl_repo")

import numpy as np
import ml_dtypes

import concourse.bacc as bacc
import concourse.tile as tile
from concourse import mybir
from concourse.bass_utils import run_bass_kernel_spmd

F32R = mybir.dt.float32r
F32 = mybir.dt.float32
BF16 = mybir.dt.bfloat16
AF = mybir.ActivationFunctionType
ALU = mybir.AluOpType
BF16NP = ml_dtypes.bfloat16

DIM, CTX_DIM, HEADS, DHEAD = 1024, 768, 16, 64
NCH, CCH = DIM // 128, CTX_DIM // 128          # 8, 6
TO, TF, NTT = 512, 2048, 4                     # own tokens, full tokens, tiles
NCTX = 77
NCTXP = 80                                     # padded for f32r even-N rule
F2, F1 = 8192, 4096                            # GEGLU proj, inner
RANK = 16
N_CORES = 8


def _build(nc):
    from contextlib import ExitStack

    dram = {}

    def din(name, shape, dt=F32R):
        dram[name] = nc.dram_tensor(name, shape, dt, kind="ExternalInput")
        return dram[name]

    xT = din("xT", [DIM, TF])
    ctxT = din("ctxT", [CTX_DIM, NCTXP], BF16)
    consts = din("consts", [128, 129])            # cols 0:128 = 1, col 128 = 1/1024
    constsb = din("constsb", [128, 256], BF16)    # all ones
    w_q = din("w_q", [DIM, DIM], BF16); w_k = din("w_k", [DIM, DIM], BF16)
    w_v = din("w_v", [DIM, DIM], BF16); w_o = din("w_o", [DIM, DIM], BF16)
    din("b_o", [DIM], F32)
    d_qkv = din("d_qkv", [DIM, 96], BF16)               # q@0:16, k@32:48, v@64:80
    u_q = din("u_q", [RANK, DIM], BF16); u_k = din("u_k", [RANK, DIM], BF16)
    u_v = din("u_v", [RANK, DIM], BF16)
    d_o = din("d_o", [DIM, RANK], BF16); u_o = din("u_o", [RANK, DIM], BF16)
    w_q2 = din("w_q2", [DIM, DIM], BF16)
    w_k2 = din("w_k2", [CTX_DIM, DIM], BF16); w_v2 = din("w_v2", [CTX_DIM, DIM], BF16)
    w_o2 = din("w_o2", [DIM, DIM], BF16); din("b_o2", [DIM], F32)
    d_q2 = din("d_q2", [DIM, RANK], BF16); u_q2 = din("u_q2", [RANK, DIM], BF16)
    d_kv2 = din("d_kv2", [CTX_DIM, 48], BF16)           # k2@0:16, v2@32:48
    u_k2 = din("u_k2", [RANK, DIM], BF16); u_v2 = din("u_v2", [RANK, DIM], BF16)
    d_o2 = din("d_o2", [DIM, RANK], BF16); u_o2 = din("u_o2", [RANK, DIM], BF16)
    for nm in ("ln1g", "ln1b", "ln2g", "ln2b", "ln3g", "ln3b"):
        din(nm, [DIM], F32)
    w_p = din("w_p", [DIM, F2], BF16); b_p = din("b_p", [F2], F32)
    d_p = din("d_p", [DIM, RANK], BF16); u_p = din("u_p", [RANK, F2], BF16)
    w_2 = din("w_2", [F1, DIM], BF16); din("b_2", [DIM], F32)
    d_f = din("d_f", [F1, RANK], BF16); u_f = din("u_f", [RANK, DIM], BF16)
    outT = nc.dram_tensor("outT", [DIM, TO], F32, kind="ExternalOutput")

    def chunked(t):
        # [C*128, N] dram -> [128, C, N] access pattern
        return t.rearrange("(c p) n -> p c n", p=128)

    with tile.TileContext(nc) as tc, \
            nc.allow_low_precision(reason="f32r/bf16 kernel by design"), \
            ExitStack() as ctx:
        cst = ctx.enter_context(tc.tile_pool(name="cst", bufs=1))
        sb = ctx.enter_context(tc.tile_pool(name="sb", bufs=1))

        # ---------------- constants ----------------
        o_col = cst.tile([128, 1], F32R, tag="o_col")       # 1/1024
        nc.sync.dma_start(out=o_col, in_=consts[:, 128:129])
        o_row = cst.tile([1, 128], F32R, tag="o_row")       # ones row (bcast lhsT)
        nc.sync.dma_start(out=o_row, in_=consts[0:1, 0:128])
        ones64 = cst.tile([65, 64], F32R, tag="ones64")     # row 64 = ones
        nc.sync.dma_start(out=ones64[64:65, :], in_=consts[0:1, 0:64])
        eps_t = cst.tile([1, 1], F32, tag="eps")
        nc.vector.memset(eps_t, 1e-5)

        lnp = {}
        for nm in ("ln1g", "ln1b", "ln2g", "ln2b", "ln3g", "ln3b",
                   "b_o", "b_o2", "b_2"):
            t = cst.tile([128, NCH], F32, tag=nm)
            nc.sync.dma_start(out=t, in_=dram[nm].rearrange("(c p) -> p c", p=128))
            lnp[nm] = t
        bp_t = cst.tile([128, F2 // 128], F32, tag="bp")
        nc.sync.dma_start(out=bp_t, in_=b_p.rearrange("(c p) -> p c", p=128))

        d_qkv_t = cst.tile([128, NCH, 96], BF16, tag="d_qkv")
        nc.sync.dma_start(out=d_qkv_t, in_=chunked(d_qkv))
        d_o_t = cst.tile([128, NCH, RANK], BF16, tag="d_o")
        nc.sync.dma_start(out=d_o_t, in_=chunked(d_o))
        d_q2_t = cst.tile([128, NCH, RANK], BF16, tag="d_q2")
        nc.sync.dma_start(out=d_q2_t, in_=chunked(d_q2))
        d_kv2_t = cst.tile([128, CCH, 48], BF16, tag="d_kv2")
        nc.sync.dma_start(out=d_kv2_t, in_=chunked(d_kv2))
        d_o2_t = cst.tile([128, NCH, RANK], BF16, tag="d_o2")
        nc.sync.dma_start(out=d_o2_t, in_=chunked(d_o2))
        d_p_t = cst.tile([128, NCH, RANK], BF16, tag="d_p")
        nc.sync.dma_start(out=d_p_t, in_=chunked(d_p))
        d_f_t = cst.tile([128, F1 // 128, RANK], BF16, tag="d_f")
        nc.sync.dma_start(out=d_f_t, in_=chunked(d_f))

        # ---------------- big persistent tiles (tag-recycled) ----------------
        K_all = sb.tile([128, NCH, TF], BF16, tag="kx")      # K^T, later h_lo
        V_all = sb.tile([128, 16, 16 * 65], BF16, tag="vx")  # V (65-padded), later h_hi
        Q_all = sb.tile([128, NCH, TO], BF16, tag="qx")      # Q^T, later Q2
        O_all = sb.tile([128, NCH, TO], BF16, tag="ox")      # O^T, later O2

        # ones columns of V (col 64 of every 65-block)
        nc.sync.dma_start(
            out=V_all.rearrange("p n (h e) -> p (n h) e", e=65)[:, :, 64:65],
            in_=constsb[:, 0:256].rearrange("p (a b) -> p a b", b=1))

        # ---------------- helpers ----------------
        def ln_stats(ps_pool, src, n_src):
            """mean/rstd over partitions via ones-matmuls. src[c]: [128, T] f32r."""
            T = src[0].shape[-1]
            m_ps = ps_pool.tile([1, T], F32, tag="st", bufs=2)
            m2_ps = ps_pool.tile([1, T], F32, tag="st", bufs=2)
            for c in range(n_src):
                xsq = sb.tile([128, T], F32R, tag="xsq", bufs=2)
                nc.scalar.activation(out=xsq, in_=src[c], func=AF.Square)
                nc.tensor.matmul(m_ps, o_col, src[c], start=(c == 0),
                                 stop=(c == n_src - 1))
                nc.tensor.matmul(m2_ps, o_col, xsq, start=(c == 0),
                                 stop=(c == n_src - 1))
            m_sb = sb.tile([1, T], F32R, tag="sst", bufs=4)
            nc.scalar.activation(out=m_sb, in_=m_ps, func=AF.Copy)
            msq = sb.tile([1, T], F32, tag="sst", bufs=4)
            nc.vector.tensor_mul(out=msq, in0=m_sb, in1=m_sb)
            var = sb.tile([1, T], F32, tag="sst", bufs=4)
            nc.vector.tensor_sub(out=var, in0=m2_ps, in1=msq)
            nc.scalar.activation(out=var, in_=var, func=AF.Sqrt, bias=eps_t)
            rstd = sb.tile([1, T], F32R, tag="sst", bufs=4)
            nc.vector.reciprocal(out=rstd, in_=var)
            return m_sb, rstd

        def ln_apply(ps_pool, src, dst, m_sb, rstd, gkey, bkey, n_src):
            T = src[0].shape[-1]
            m_bc = ps_pool.tile([128, T], F32, tag="bc", bufs=2)
            nc.tensor.matmul(m_bc, o_row, m_sb, start=True, stop=True)
            r_bc = ps_pool.tile([128, T], F32, tag="bc", bufs=2)
            nc.tensor.matmul(r_bc, o_row, rstd, start=True, stop=True)
            g_t, b_t = lnp[gkey], lnp[bkey]
            for c in range(n_src):
                nc.vector.tensor_sub(out=dst[c], in0=src[c], in1=m_bc)
                nc.vector.tensor_mul(out=dst[c], in0=dst[c], in1=r_bc)
                nc.vector.tensor_scalar(out=dst[c], in0=dst[c],
                                        scalar1=g_t[:, c:c + 1],
                                        scalar2=b_t[:, c:c + 1],
                                        op0=ALU.mult, op1=ALU.add)

        def u_slice(u_dram, lo, hi, dt=BF16, row0=0):
            t = sb.tile([row0 + RANK, hi - lo], dt, tag="ust", bufs=2,
                        name="ust")
            nc.sync.dma_start(out=t[row0:row0 + RANK, :], in_=u_dram[:, lo:hi])
            return t[row0:row0 + RANK, :]

        def proj_T(ps_pool, w_dram, src, out_write, lora, n_c=NCH, dt=BF16,
                   n_free=TO, lrow=0):
            """out^T[ic] = sum_c W[c,ic].T @ src[c] + lora up. out_write(ic, ps)."""
            u_dram, dn_rhs, ldt = lora
            for h in range(2):
                pss = [ps_pool.tile([128, n_free], F32, tag="pj", bufs=4,
                                    name=f"pj{h}_{i}") for i in range(4)]
                for c in range(n_c):
                    wt = sb.tile([128, 512], dt, tag="wst", bufs=2)
                    nc.sync.dma_start(out=wt,
                                      in_=chunked(w_dram)[:, c, h * 512:(h + 1) * 512])
                    for i in range(4):
                        nc.tensor.matmul(pss[i], wt[:, i * 128:(i + 1) * 128],
                                         src[c], start=(c == 0), stop=False)
                for i in range(4):
                    ic = 4 * h + i
                    ut = u_slice(u_dram, ic * 128, (ic + 1) * 128, ldt,
                                 row0=lrow)
                    nc.tensor.matmul(pss[i], ut, dn_rhs, start=False, stop=True)
                    out_write(ic, pss[i])

        def attn(ps_pool, q_all, k_all, v_all, n_k_chunks, n_k, o_all):
            """q_all [128, NCH, TO]; k_all [128, NCH, n_k]; v_all(tc) -> V tile."""
            for hd in range(HEADS):
                ic, p0 = hd // 2, 64 * (hd % 2)
                av = ps_pool.tile([65, TO], F32, tag="av", bufs=2)
                for tcc in range(n_k_chunks):
                    k_lo = tcc * 128
                    k_n = min(128, n_k - k_lo)
                    s_ps = ps_pool.tile([k_n, TO], F32, tag="sx", bufs=3)
                    nc.tensor.matmul(
                        s_ps, k_all[p0:p0 + 64, ic, k_lo:k_lo + k_n],
                        q_all[p0:p0 + 64, ic, :], start=True, stop=True)
                    es = sb.tile([k_n, TO], BF16, tag="es", bufs=2)
                    nc.scalar.activation(out=es, in_=s_ps, func=AF.Exp,
                                         scale=float(DHEAD) ** -0.5)
                    nc.tensor.matmul(av, v_all(tcc)[0:k_n, hd * 65:(hd + 1) * 65],
                                     es, start=(tcc == 0),
                                     stop=(tcc == n_k_chunks - 1))
                den = sb.tile([65, TO], F32R, tag="den", bufs=1)
                nc.scalar.activation(out=den[64:65, :], in_=av[64:65, :],
                                     func=AF.Copy)
                nc.vector.reciprocal(out=den[64:65, :], in_=den[64:65, :])
                rd = ps_pool.tile([64, TO], F32, tag="rd", bufs=2)
                nc.tensor.matmul(rd, ones64[64:65, :], den[64:65, :],
                                 start=True, stop=True)
                o_cp = sb.tile([64, TO], F32, tag="cp", bufs=3)
                nc.scalar.activation(out=o_cp, in_=av[0:64, :], func=AF.Copy)
                if p0 == 0:
                    nc.vector.tensor_mul(out=o_all[0:64, ic, :], in0=o_cp, in1=rd)
                else:
                    o_tmp = sb.tile([64, TO], BF16, tag="ot", bufs=1)
                    nc.vector.tensor_mul(out=o_tmp, in0=o_cp, in1=rd)
                    nc.sync.dma_start(out=o_all[p0:p0 + 64, ic, :], in_=o_tmp)

        def wo_phase(ps_pool, w_dram, o_all, d_t, u_dram, bias_key, x_res, x2_dst):
            """x2 = x_res + W_o.T@O + u_o.T@(d_o.T@O) + b_o   (bf16 weights)."""
            od_ps = ps_pool.tile([RANK, TO], F32, tag="pj", bufs=4)
            for c in range(NCH):
                nc.tensor.matmul(od_ps, d_t[:, c, :], o_all[:, c, :],
                                 start=(c == 0), stop=(c == NCH - 1))
            od_sb = sb.tile([RANK, TO], BF16, tag="odx", bufs=2)
            nc.scalar.activation(out=od_sb, in_=od_ps, func=AF.Copy)
            for h in range(2):
                pss = [ps_pool.tile([128, TO], F32, tag="pj", bufs=4,
                                    name=f"pjo{h}_{i}") for i in range(4)]
                for c in range(NCH):
                    wt = sb.tile([128, 512], BF16, tag="wst", bufs=2)
                    nc.sync.dma_start(out=wt,
                                      in_=chunked(w_dram)[:, c, h * 512:(h + 1) * 512])
                    for i in range(4):
                        nc.tensor.matmul(pss[i], wt[:, i * 128:(i + 1) * 128],
                                         o_all[:, c, :], start=(c == 0), stop=False)
                for i in range(4):
                    dc = 4 * h + i
                    ut = u_slice(u_dram, dc * 128, (dc + 1) * 128, BF16)
                    nc.tensor.matmul(pss[i], ut, od_sb, start=False, stop=True)
                    t = sb.tile([128, TO], F32, tag="cp", bufs=3)
                    nc.scalar.activation(out=t, in_=pss[i], func=AF.Identity,
                                         bias=lnp[bias_key][:, dc:dc + 1])
                    nc.vector.tensor_add(out=x2_dst[:, dc, :], in0=t,
                                         in1=x_res(dc))

        # ======================= phase A: LN1 + Q/K/V =======================

        with tc.tile_pool(name="psA", bufs=1, space="PSUM") as psA:
            for tt in range(NTT):
                x_tt = sb.tile([128, NCH, TO], F32R, tag="xs", bufs=1)
                nc.sync.dma_start(out=x_tt,
                                  in_=chunked(xT)[:, :, tt * TO:(tt + 1) * TO])
                xs = [x_tt[:, c, :] for c in range(NCH)]
                m_sb, rstd = ln_stats(psA, xs, NCH)
                h1 = sb.tile([128, NCH, TO], BF16, tag="h1", bufs=1)
                h1c = [h1[:, c, :] for c in range(NCH)]
                ln_apply(psA, xs, h1c, m_sb, rstd, "ln1g", "ln1b", NCH)

                # packed qkv lora down: [96, TO]
                xd_ps = psA.tile([96, TO], F32, tag="pj", bufs=4)
                for c in range(NCH):
                    nc.tensor.matmul(xd_ps, d_qkv_t[:, c, :], h1c[c],
                                     start=(c == 0), stop=(c == NCH - 1))
                xd_tt = sb.tile([96, TO], BF16, tag="xd", bufs=2)
                nc.scalar.activation(out=xd_tt, in_=xd_ps, func=AF.Copy)

                if tt == 0:
                    def wq_out(ic, ps):
                        nc.scalar.activation(out=Q_all[:, ic, :], in_=ps,
                                             func=AF.Copy)
                    proj_T(psA, w_q, h1c, wq_out,
                           lora=(u_q, xd_tt[0:16, :], BF16))

                def wk_out(ic, ps, _tt=tt):
                    nc.scalar.activation(
                        out=K_all[:, ic, _tt * TO:(_tt + 1) * TO], in_=ps,
                        func=AF.Copy)
                proj_T(psA, w_k, h1c, wk_out,
                       lora=(u_k, xd_tt[32:48, :], BF16), lrow=32)

                # V natural layout, 65-padded heads
                for half in range(2):
                    ps_v = [psA.tile([128, TO], F32, tag="pj", bufs=4,
                                     name=f"psv{half}_{i}") for i in range(4)]
                    for c in range(NCH):
                        wt = sb.tile([128, 512], BF16, tag="wst", bufs=2)
                        nc.sync.dma_start(
                            out=wt,
                            in_=chunked(w_v)[:, c, half * 512:(half + 1) * 512])
                        for tcc in range(4):
                            nc.tensor.matmul(
                                ps_v[tcc], h1[:, c, tcc * 128:(tcc + 1) * 128],
                                wt, start=(c == 0), stop=False)
                    ut = u_slice(u_v, half * 512, (half + 1) * 512, row0=64)
                    for tcc in range(4):
                        nc.tensor.matmul(
                            ps_v[tcc],
                            xd_tt[64:80, tcc * 128:(tcc + 1) * 128],
                            ut, start=False, stop=True)
                        vtile = V_all[:, tt * 4 + tcc, :] \
                            .rearrange("p (h e) -> p h e", e=65)
                        nc.scalar.activation(
                            out=vtile[:, half * 8:(half + 1) * 8, 0:64],
                            in_=ps_v[tcc].rearrange("p (h e) -> p h e", e=64),
                            func=AF.Copy)

        # ======================= phase B: self-attention ====================
        with tc.tile_pool(name="psB", bufs=1, space="PSUM") as psB:
            attn(psB, Q_all, K_all, lambda tcc: V_all[:, tcc, :], TF // 128, TF,
                 O_all)

        # ======================= phase C: Wo + residual =====================
        x2_all = sb.tile([128, NCH, TO], F32R, tag="x2")
        with tc.tile_pool(name="psC", bufs=1, space="PSUM") as psC:
            xres = sb.tile([128, NCH, TO], F32R, tag="xs", bufs=1)
            nc.sync.dma_start(out=xres, in_=chunked(xT)[:, :, 0:TO])
            wo_phase(psC, w_o, O_all, d_o_t, u_o, "b_o",
                     lambda dc: xres[:, dc, :], x2_all)

        # ============== phase D: LN2, cross-attention =======================
        Q2_all = sb.tile([128, NCH, TO], BF16, tag="qx")
        K2_all = sb.tile([128, NCH, NCTXP], BF16, tag="k2")
        V2_t = sb.tile([128, 16 * 65], BF16, tag="v2")
        nc.sync.dma_start(
            out=V2_t.rearrange("p (h e) -> p h e", e=65)[:, :, 64:65],
            in_=constsb[:, 0:16].rearrange("p (a b) -> p a b", b=1))
        ctx_t = sb.tile([128, CCH, NCTXP], BF16, tag="ctx")
        nc.sync.dma_start(out=ctx_t, in_=chunked(ctxT))
        O2_all = sb.tile([128, NCH, TO], BF16, tag="ox")

        with tc.tile_pool(name="psD", bufs=1, space="PSUM") as psD:
            x2c = [x2_all[:, c, :] for c in range(NCH)]
            m_sb, rstd = ln_stats(psD, x2c, NCH)
            h2 = sb.tile([128, NCH, TO], BF16, tag="h1", bufs=1)
            h2c = [h2[:, c, :] for c in range(NCH)]
            ln_apply(psD, x2c, h2c, m_sb, rstd, "ln2g", "ln2b", NCH)

            q2d_ps = psD.tile([RANK, TO], F32, tag="pj", bufs=4)
            for c in range(NCH):
                nc.tensor.matmul(q2d_ps, d_q2_t[:, c, :], h2c[c],
                                 start=(c == 0), stop=(c == NCH - 1))
            q2d_sb = sb.tile([RANK, TO], BF16, tag="odx", bufs=2)
            nc.scalar.activation(out=q2d_sb, in_=q2d_ps, func=AF.Copy)

            def wq2_out(ic, ps):
                nc.scalar.activation(out=Q2_all[:, ic, :], in_=ps, func=AF.Copy)
            proj_T(psD, w_q2, h2c, wq2_out, lora=(u_q2, q2d_sb, BF16))

            # kv2 lora down from raw context
            cd_ps = psD.tile([48, NCTXP], F32, tag="pj", bufs=4)
            for c in range(CCH):
                nc.tensor.matmul(cd_ps, d_kv2_t[:, c, :], ctx_t[:, c, :],
                                 start=(c == 0), stop=(c == CCH - 1))
            cd_sb = sb.tile([48, NCTXP], BF16, tag="odx", bufs=2)
            nc.scalar.activation(out=cd_sb, in_=cd_ps, func=AF.Copy)

            # K2^T
            for h in range(2):
                pss = [psD.tile([128, NCTXP], F32, tag="pj", bufs=4,
                                name=f"pk2{h}_{i}") for i in range(4)]
                for c in range(CCH):
                    wt = sb.tile([128, 512], BF16, tag="wst", bufs=2)
                    nc.sync.dma_start(
                        out=wt, in_=chunked(w_k2)[:, c, h * 512:(h + 1) * 512])
                    for i in range(4):
                        nc.tensor.matmul(pss[i], wt[:, i * 128:(i + 1) * 128],
                                         ctx_t[:, c, :], start=(c == 0), stop=False)
                for i in range(4):
                    ic = 4 * h + i
                    ut = u_slice(u_k2, ic * 128, (ic + 1) * 128)
                    nc.tensor.matmul(pss[i], ut, cd_sb[0:16, :], start=False,
                                     stop=True)
                    nc.scalar.activation(out=K2_all[:, ic, :], in_=pss[i],
                                         func=AF.Copy)
            # V2 natural
            for half in range(2):
                ps_v = psD.tile([NCTX, 512], F32, tag="pj", bufs=4)
                for c in range(CCH):
                    wt = sb.tile([128, 512], BF16, tag="wst", bufs=2)
                    nc.sync.dma_start(
                        out=wt, in_=chunked(w_v2)[:, c, half * 512:(half + 1) * 512])
                    nc.tensor.matmul(ps_v, ctx_t[:, c, 0:NCTX], wt,
                                     start=(c == 0), stop=False)
                ut = u_slice(u_v2, half * 512, (half + 1) * 512, row0=32)
                nc.tensor.matmul(ps_v, cd_sb[32:48, 0:NCTX], ut, start=False, stop=True)
                nc.scalar.activation(
                    out=V2_t.rearrange("p (h e) -> p h e", e=65)
                        [0:NCTX, half * 8:(half + 1) * 8, 0:64],
                    in_=ps_v.rearrange("p (h e) -> p h e", e=64),
                    func=AF.Copy)

        with tc.tile_pool(name="psD2", bufs=1, space="PSUM") as psD2:
            attn(psD2, Q2_all, K2_all, lambda tcc: V2_t, 1, NCTX, O2_all)

        # ======================= phase E: Wo2 + residual ====================
        x3_all = sb.tile([128, NCH, TO], F32R, tag="x3")
        with tc.tile_pool(name="psE", bufs=1, space="PSUM") as psE:
            wo_phase(psE, w_o2, O2_all, d_o2_t, u_o2, "b_o2",
                     lambda dc: x2_all[:, dc, :], x3_all)

        # ======================= phase F: LN3 ===============================
        h3 = sb.tile([128, NCH, TO], BF16, tag="h1", bufs=1)
        with tc.tile_pool(name="psF", bufs=1, space="PSUM") as psF:
            x3c = [x3_all[:, c, :] for c in range(NCH)]
            m_sb, rstd = ln_stats(psF, x3c, NCH)
            h3c = [h3[:, c, :] for c in range(NCH)]
            ln_apply(psF, x3c, h3c, m_sb, rstd, "ln3g", "ln3b", NCH)

        # ======================= phase G: GEGLU FF ==========================
        h_lo = sb.tile([128, 16, TO], BF16, tag="kx")   # h chunks 0..15
        h_hi = sb.tile([128, 16, TO], BF16, tag="vx")   # h chunks 16..31
        with tc.tile_pool(name="psG", bufs=1, space="PSUM") as psG:
            h3c = [h3[:, c, :] for c in range(NCH)]
            pd_ps = psG.tile([RANK, TO], F32, tag="pj", bufs=4)
            for c in range(NCH):
                nc.tensor.matmul(pd_ps, d_p_t[:, c, :], h3c[c],
                                 start=(c == 0), stop=(c == NCH - 1))
            pd_sb = sb.tile([RANK, TO], BF16, tag="odx", bufs=2)
            nc.scalar.activation(out=pd_sb, in_=pd_ps, func=AF.Copy)

            def proj_block(j, gate):
                """proj^T chunks for col block j (4 chunks of 128).
                Returns [(fc, psum_tile)]."""
                base = (32 if gate else 0) + 4 * j
                pss = [psG.tile([128, TO], F32, tag="pj", bufs=4,
                                name=f"pg{h}_{i}") for i in range(4)]
                for c in range(NCH):
                    wt = sb.tile([128, 512], BF16, tag="wst", bufs=2)
                    nc.sync.dma_start(
                        out=wt,
                        in_=chunked(w_p)[:, c, base * 128:(base + 4) * 128])
                    for i in range(4):
                        nc.tensor.matmul(pss[i], wt[:, i * 128:(i + 1) * 128],
                                         h3c[c], start=(c == 0), stop=False)
                out = []
                for i in range(4):
                    fc = base + i
                    ut = u_slice(u_p, fc * 128, (fc + 1) * 128)
                    nc.tensor.matmul(pss[i], ut, pd_sb, start=False, stop=True)
                    out.append((fc, pss[i]))
                return out

            def h_ap(fc):
                return h_lo[:, fc, :] if fc < 16 else h_hi[:, fc - 16, :]

            for j in range(8):
                gels = []
                for fc, ps in proj_block(j, gate=True):
                    gel = sb.tile([128, TO], F32, tag="gel", bufs=4)
                    nc.scalar.activation(out=gel, in_=ps, func=AF.Gelu,
                                         bias=bp_t[:, fc:fc + 1])
                    gels.append(gel)
                for idx, (fc, ps) in enumerate(proj_block(j, gate=False)):
                    t = sb.tile([128, TO], F32, tag="cp", bufs=3)
                    nc.vector.tensor_scalar_add(out=t, in0=ps,
                                                scalar1=bp_t[:, fc:fc + 1])
                    nc.vector.tensor_mul(out=h_ap(fc), in0=t, in1=gels[idx])

            # second FF matmul + bias + residual -> outT
            hd_ps = psG.tile([RANK, TO], F32, tag="pj", bufs=4)
            for fc in range(32):
                nc.tensor.matmul(hd_ps, d_f_t[:, fc, :], h_ap(fc),
                                 start=(fc == 0), stop=(fc == 31))
            hd_sb = sb.tile([RANK, TO], BF16, tag="odx", bufs=2)
            nc.scalar.activation(out=hd_sb, in_=hd_ps, func=AF.Copy)
            for h in range(2):
                pss = [psG.tile([128, TO], F32, tag="pj", bufs=4,
                                name=f"pg{h}_{i}") for i in range(4)]
                for fc in range(32):
                    wt = sb.tile([128, 512], BF16, tag="wst", bufs=2)
                    nc.sync.dma_start(
                        out=wt, in_=chunked(w_2)[:, fc, h * 512:(h + 1) * 512])
                    for i in range(4):
                        nc.tensor.matmul(pss[i], wt[:, i * 128:(i + 1) * 128],
                                         h_ap(fc), start=(fc == 0), stop=False)
                for i in range(4):
                    dc = 4 * h + i
                    ut = u_slice(u_f, dc * 128, (dc + 1) * 128)
                    nc.tensor.matmul(pss[i], ut, hd_sb, start=False, stop=True)
                    t = sb.tile([128, TO], F32, tag="cp", bufs=3)
                    nc.scalar.activation(out=t, in_=pss[i], func=AF.Identity,
                                         bias=lnp["b_2"][:, dc:dc + 1])
                    of = sb.tile([128, TO], F32, tag="gel", bufs=4)
                    nc.vector.tensor_add(out=of, in0=t, in1=x3_all[:, dc, :])
                    nc.sync.dma_start(out=outT[dc * 128:(dc + 1) * 128, :], in_=of)

    nc.finalize()
    return nc


_CACHE = {}


def _get_nc():
    if "nc" not in _CACHE:
        _CACHE["nc"] = _build(bacc.Bacc())
    return _CACHE["nc"]


def _prep_in_maps(x, context, params):
    p = params
    sc = lambda a: float(a) * 1.0 / RANK  # LORA_W * alpha / rank

    def f32(a):
        return np.ascontiguousarray(np.asarray(a, dtype=np.float32))

    def bf(a):
        return np.ascontiguousarray(np.asarray(a, dtype=np.float32)
                                    .astype(BF16NP))

    a1, a2, ff = p["attn1"], p["attn2"], p["ff"]
    shared = {
        "consts": np.concatenate(
            [np.ones((128, 128), np.float32),
             np.full((128, 1), 1.0 / DIM, np.float32)], 1),
        "constsb": np.ones((128, 256), np.float32).astype(BF16NP),
        "w_q": bf(a1["Wq"]), "w_k": bf(a1["Wk"]), "w_v": bf(a1["Wv"]),
        "w_o": bf(a1["Wo"]), "b_o": f32(a1["bo"]),
        "u_q": bf(np.asarray(a1["qu"]) * sc(a1["qa"])),
        "u_k": bf(np.asarray(a1["ku"]) * sc(a1["ka"])),
        "u_v": bf(np.asarray(a1["vu"]) * sc(a1["va"])),
        "d_o": bf(a1["od"]), "u_o": bf(np.asarray(a1["ou"]) * sc(a1["oa"])),
        "w_q2": bf(a2["Wq"]), "w_k2": bf(a2["Wk"]), "w_v2": bf(a2["Wv"]),
        "w_o2": bf(a2["Wo"]), "b_o2": f32(a2["bo"]),
        "d_q2": bf(a2["qd"]), "u_q2": bf(np.asarray(a2["qu"]) * sc(a2["qa"])),
        "u_k2": bf(np.asarray(a2["ku"]) * sc(a2["ka"])),
        "u_v2": bf(np.asarray(a2["vu"]) * sc(a2["va"])),
        "d_o2": bf(a2["od"]), "u_o2": bf(np.asarray(a2["ou"]) * sc(a2["oa"])),
        "ln1g": f32(p["ln1_g"]), "ln1b": f32(p["ln1_b"]),
        "ln2g": f32(p["ln2_g"]), "ln2b": f32(p["ln2_b"]),
        "ln3g": f32(p["ln3_g"]), "ln3b": f32(p["ln3_b"]),
        "w_p": bf(ff["Wp"]), "b_p": f32(ff["bp"]),
        "d_p": bf(ff["pd"]), "u_p": bf(np.asarray(ff["pu"]) * sc(ff["pa"])),
        "w_2": bf(ff["W2"]), "b_2": f32(ff["b2"]),
        "d_f": bf(ff["fd"]), "u_f": bf(np.asarray(ff["fu"]) * sc(ff["fa"])),
    }
    d_qkv = np.zeros((DIM, 96), BF16NP)
    d_qkv[:, 0:16] = np.asarray(a1["qd"]); d_qkv[:, 32:48] = np.asarray(a1["kd"])
    d_qkv[:, 64:80] = np.asarray(a1["vd"])
    shared["d_qkv"] = d_qkv
    d_kv2 = np.zeros((CTX_DIM, 48), BF16NP)
    d_kv2[:, 0:16] = np.asarray(a2["kd"]); d_kv2[:, 32:48] = np.asarray(a2["vd"])
    shared["d_kv2"] = d_kv2

    x = np.asarray(x, np.float32)
    context = np.asarray(context, np.float32)
    in_maps = []
    for core in range(N_CORES):
        b, t0 = core // 4, (core % 4) * TO
        xt = x[b].T  # [DIM, TF]
        m = dict(shared)
        m["xT"] = np.ascontiguousarray(
            np.concatenate([xt[:, t0:], xt[:, :t0]], axis=1))
        ctp = np.zeros((CTX_DIM, NCTXP), BF16NP)
        ctp[:, :NCTX] = context[b].T.astype(BF16NP)
        m["ctxT"] = ctp
        in_maps.append(m)
    return in_maps


def run_spmd(in_maps, **kw):
    return run_bass_kernel_spmd(_get_nc(), in_maps,
                                core_ids=list(range(N_CORES)), **kw)


def kernel(x, context, params):
    in_maps = _prep_in_maps(x, context, params)
    res = run_spmd(in_maps)
    B, N = np.asarray(x).shape[:2]
    out = np.empty((B, N, DIM), np.float32)
    for core in range(N_CORES):
        b, t0 = core // 4, (core % 4) * TO
        out[b, t0:t0 + TO, :] = res.results[core]["outT"].T
    return out


# revision 9
# speedup vs baseline: 1.4436x; 1.2749x over previous
"""Trainium2 Bass kernel for a BasicTransformerBlock (self-attn + cross-attn +
GEGLU FF, all with rank-16 LoRA deltas), sharded over 8 NeuronCores.

Sharding: core c handles batch b = c//4 and a 512-token quarter of the 2048
token sequence. Each core's x^T input is rotated so its own tokens are columns
0:512; self-attention K/V are computed for the full (rotated) sequence, which
is valid because softmax attention is permutation-invariant over keys.

Layout: activations are kept transposed (model dim on partitions, tokens on
the free axis). LayerNorm reductions over the model dim use ones-vector
matmuls; softmax denominators come from a ones-column appended to V; per-token
broadcasts use K=1 matmuls.

Dtypes: float32r (fp32 storage, reduced-precision PE multiply, full-rate) for
projections/FF/LN; bf16 for attention internals (Q/K/V/exp(S)/O and the
attention output projection) to fit SBUF.
"""

import sys

sys.path.insert(0, "/opt/trn_rl_repo")

import numpy as np
import ml_dtypes

import concourse.bacc as bacc
import concourse.tile as tile
from concourse import mybir
from concourse.bass_utils import run_bass_kernel_spmd

F32R = mybir.dt.float32r
F32 = mybir.dt.float32
BF16 = mybir.dt.bfloat16
AF = mybir.ActivationFunctionType
ALU = mybir.AluOpType
BF16NP = ml_dtypes.bfloat16

DIM, CTX_DIM, HEADS, DHEAD = 1024, 768, 16, 64
NCH, CCH = DIM // 128, CTX_DIM // 128          # 8, 6
TO, TF, NTT = 512, 2048, 4                     # own tokens, full tokens, tiles
NCTX = 77
NCTXP = 80                                     # padded for f32r even-N rule
F2, F1 = 8192, 4096                            # GEGLU proj, inner
RANK = 16
N_CORES = 8


def _build(nc):
    from contextlib import ExitStack

    dram = {}

    def din(name, shape, dt=F32R):
        dram[name] = nc.dram_tensor(name, shape, dt, kind="ExternalInput")
        return dram[name]

    xT = din("xT", [DIM, TF])
    ctxT = din("ctxT", [CTX_DIM, NCTXP], BF16)
    consts = din("consts", [128, 129])            # cols 0:128 = 1, col 128 = 1/1024
    constsb = din("constsb", [128, 256], BF16)    # all ones
    w_q = din("w_q", [DIM, DIM], BF16); w_k = din("w_k", [DIM, DIM], BF16)
    w_v = din("w_v", [DIM, DIM], BF16); w_o = din("w_o", [DIM, DIM], BF16)
    din("b_o", [DIM], F32)
    d_qkv = din("d_qkv", [DIM, 96], BF16)               # q@0:16, k@32:48, v@64:80
    u_q = din("u_q", [RANK, DIM], BF16); u_k = din("u_k", [RANK, DIM], BF16)
    u_v = din("u_v", [RANK, DIM], BF16)
    d_o = din("d_o", [DIM, RANK], BF16); u_o = din("u_o", [RANK, DIM], BF16)
    w_q2 = din("w_q2", [DIM, DIM], BF16)
    w_k2 = din("w_k2", [CTX_DIM, DIM], BF16); w_v2 = din("w_v2", [CTX_DIM, DIM], BF16)
    w_o2 = din("w_o2", [DIM, DIM], BF16); din("b_o2", [DIM], F32)
    d_q2 = din("d_q2", [DIM, RANK], BF16); u_q2 = din("u_q2", [RANK, DIM], BF16)
    d_kv2 = din("d_kv2", [CTX_DIM, 48], BF16)           # k2@0:16, v2@32:48
    u_k2 = din("u_k2", [RANK, DIM], BF16); u_v2 = din("u_v2", [RANK, DIM], BF16)
    d_o2 = din("d_o2", [DIM, RANK], BF16); u_o2 = din("u_o2", [RANK, DIM], BF16)
    for nm in ("ln1g", "ln1b", "ln2g", "ln2b", "ln3g", "ln3b"):
        din(nm, [DIM], F32)
    w_p = din("w_p", [DIM, F2], BF16); b_p = din("b_p", [F2], F32)
    d_p = din("d_p", [DIM, RANK], BF16); u_p = din("u_p", [RANK, F2], BF16)
    w_2 = din("w_2", [F1, DIM], BF16); din("b_2", [DIM], F32)
    d_f = din("d_f", [F1, RANK], BF16); u_f = din("u_f", [RANK, DIM], BF16)
    outT = nc.dram_tensor("outT", [DIM, TO], F32, kind="ExternalOutput")

    def chunked(t):
        # [C*128, N] dram -> [128, C, N] access pattern
        return t.rearrange("(c p) n -> p c n", p=128)

    with tile.TileContext(nc) as tc, \
            nc.allow_low_precision(reason="f32r/bf16 kernel by design"), \
            ExitStack() as ctx:
        cst = ctx.enter_context(tc.tile_pool(name="cst", bufs=1))
        sb = ctx.enter_context(tc.tile_pool(name="sb", bufs=1))

        # ---------------- constants ----------------
        o_col = cst.tile([128, 1], F32R, tag="o_col")       # 1/1024
        nc.sync.dma_start(out=o_col, in_=consts[:, 128:129])
        o_row = cst.tile([1, 128], F32R, tag="o_row")       # ones row (bcast lhsT)
        nc.sync.dma_start(out=o_row, in_=consts[0:1, 0:128])
        ones64 = cst.tile([65, 64], F32R, tag="ones64")     # row 64 = ones
        nc.sync.dma_start(out=ones64[64:65, :], in_=consts[0:1, 0:64])
        eps_t = cst.tile([1, 1], F32, tag="eps")
        nc.vector.memset(eps_t, 1e-5)

        lnp = {}
        for nm in ("ln1g", "ln1b", "ln2g", "ln2b", "ln3g", "ln3b",
                   "b_o", "b_o2", "b_2"):
            t = cst.tile([128, NCH], F32, tag=nm)
            nc.sync.dma_start(out=t, in_=dram[nm].rearrange("(c p) -> p c", p=128))
            lnp[nm] = t
        bp_t = cst.tile([128, F2 // 128], F32, tag="bp")
        nc.sync.dma_start(out=bp_t, in_=b_p.rearrange("(c p) -> p c", p=128))

        d_qkv_t = cst.tile([128, NCH, 96], BF16, tag="d_qkv")
        nc.sync.dma_start(out=d_qkv_t, in_=chunked(d_qkv))
        d_o_t = cst.tile([128, NCH, RANK], BF16, tag="d_o")
        nc.sync.dma_start(out=d_o_t, in_=chunked(d_o))
        d_q2_t = cst.tile([128, NCH, RANK], BF16, tag="d_q2")
        nc.sync.dma_start(out=d_q2_t, in_=chunked(d_q2))
        d_kv2_t = cst.tile([128, CCH, 48], BF16, tag="d_kv2")
        nc.sync.dma_start(out=d_kv2_t, in_=chunked(d_kv2))
        d_o2_t = cst.tile([128, NCH, RANK], BF16, tag="d_o2")
        nc.sync.dma_start(out=d_o2_t, in_=chunked(d_o2))
        d_p_t = cst.tile([128, NCH, RANK], BF16, tag="d_p")
        nc.sync.dma_start(out=d_p_t, in_=chunked(d_p))
        d_f_t = cst.tile([128, F1 // 128, RANK], BF16, tag="d_f")
        nc.sync.dma_start(out=d_f_t, in_=chunked(d_f))

        # ---------------- big persistent tiles (tag-recycled) ----------------
        K_all = sb.tile([128, NCH, TF], BF16, tag="kx")      # K^T, later h_lo
        V_all = sb.tile([128, 16, 16 * 65], BF16, tag="vx")  # V (65-padded), later h_hi
        Q_all = sb.tile([128, NCH, TO], BF16, tag="qx")      # Q^T, later Q2
        O_all = sb.tile([128, NCH, TO], BF16, tag="ox")      # O^T, later O2

        # ones columns of V (col 64 of every 65-block)
        nc.sync.dma_start(
            out=V_all.rearrange("p n (h e) -> p (n h) e", e=65)[:, :, 64:65],
            in_=constsb[:, 0:256].rearrange("p (a b) -> p a b", b=1))

        # ---------------- helpers ----------------
        def ln_stats(ps_pool, src, n_src):
            """mean/rstd over partitions via ones-matmuls. src[c]: [128, T] f32r."""
            T = src[0].shape[-1]
            m_ps = ps_pool.tile([1, T], F32, tag="st", bufs=2)
            m2_ps = ps_pool.tile([1, T], F32, tag="st", bufs=2)
            for c in range(n_src):
                xsq = sb.tile([128, T], F32R, tag="xsq", bufs=2)
                nc.scalar.activation(out=xsq, in_=src[c], func=AF.Square)
                nc.tensor.matmul(m_ps, o_col, src[c], start=(c == 0),
                                 stop=(c == n_src - 1))
                nc.tensor.matmul(m2_ps, o_col, xsq, start=(c == 0),
                                 stop=(c == n_src - 1))
            m_sb = sb.tile([1, T], F32R, tag="sst", bufs=4)
            nc.vector.tensor_copy(out=m_sb, in_=m_ps)
            msq = sb.tile([1, T], F32, tag="sst", bufs=4)
            nc.vector.tensor_mul(out=msq, in0=m_sb, in1=m_sb)
            var = sb.tile([1, T], F32, tag="sst", bufs=4)
            nc.vector.tensor_sub(out=var, in0=m2_ps, in1=msq)
            sv = sb.tile([1, T], F32R, tag="sst", bufs=4)
            nc.scalar.activation(out=sv, in_=var, func=AF.Sqrt, bias=eps_t)
            return m_sb, sv

        def ln_apply(ps_pool, src, dst, m_sb, sv, gkey, bkey, n_src):
            T = src[0].shape[-1]
            m_bc = ps_pool.tile([128, T], F32, tag="bc", bufs=2)
            nc.tensor.matmul(m_bc, o_row, m_sb, start=True, stop=True)
            sv_bc = ps_pool.tile([128, T], F32, tag="bc", bufs=2)
            nc.tensor.matmul(sv_bc, o_row, sv, start=True, stop=True)
            rs_sb = sb.tile([128, T], F32, tag="rsb", bufs=2)
            nc.vector.reciprocal_approx_fast(out=rs_sb, in_=sv_bc)
            g_t, b_t = lnp[gkey], lnp[bkey]
            for c in range(n_src):
                nc.vector.tensor_sub(out=dst[c], in0=src[c], in1=m_bc)
                nc.vector.tensor_mul(out=dst[c], in0=dst[c], in1=rs_sb)
                nc.vector.tensor_scalar(out=dst[c], in0=dst[c],
                                        scalar1=g_t[:, c:c + 1],
                                        scalar2=b_t[:, c:c + 1],
                                        op0=ALU.mult, op1=ALU.add)

        def u_slice(u_dram, lo, hi, dt=BF16, row0=0):
            t = sb.tile([row0 + RANK, hi - lo], dt, tag="ust", bufs=2,
                        name="ust")
            nc.sync.dma_start(out=t[row0:row0 + RANK, :], in_=u_dram[:, lo:hi])
            return t[row0:row0 + RANK, :]

        def proj_T(ps_pool, w_dram, src, out_write, lora, n_c=NCH, dt=BF16,
                   n_free=TO, lrow=0):
            """out^T[ic] = sum_c W[c,ic].T @ src[c] + lora up. out_write(ic, ps)."""
            u_dram, dn_rhs, ldt = lora
            for h in range(2):
                pss = [ps_pool.tile([128, n_free], F32, tag="pj", bufs=4,
                                    name=f"pj{h}_{i}") for i in range(4)]
                for c in range(n_c):
                    wt = sb.tile([128, 512], dt, tag="wst", bufs=3)
                    nc.scalar.dma_start(out=wt,
                                        in_=chunked(w_dram)[:, c, h * 512:(h + 1) * 512])
                    for i in range(4):
                        nc.tensor.matmul(pss[i], wt[:, i * 128:(i + 1) * 128],
                                         src[c], start=(c == 0), stop=False)
                for i in range(4):
                    ic = 4 * h + i
                    ut = u_slice(u_dram, ic * 128, (ic + 1) * 128, ldt,
                                 row0=lrow)
                    nc.tensor.matmul(pss[i], ut, dn_rhs, start=False, stop=True)
                    out_write(ic, pss[i])

        def attn(ps_pool, q_all, k_all, v_all, n_k_chunks, n_k, o_all):
            """q_all [128, NCH, TO]; k_all [128, NCH, n_k]; v_all(tc) -> V tile."""
            for hd in range(HEADS):
                ic, p0 = hd // 2, 64 * (hd % 2)
                av = ps_pool.tile([65, TO], F32, tag="av", bufs=2)
                for tcc in range(n_k_chunks):
                    k_lo = tcc * 128
                    k_n = min(128, n_k - k_lo)
                    s_ps = ps_pool.tile([k_n, TO], F32, tag="sx", bufs=3)
                    nc.tensor.matmul(
                        s_ps, k_all[p0:p0 + 64, ic, k_lo:k_lo + k_n],
                        q_all[p0:p0 + 64, ic, :], start=True, stop=True)
                    es = sb.tile([k_n, TO], BF16, tag="es", bufs=2)
                    nc.scalar.activation(out=es, in_=s_ps, func=AF.Exp,
                                         scale=float(DHEAD) ** -0.5)
                    nc.tensor.matmul(av, v_all(tcc)[0:k_n, hd * 65:(hd + 1) * 65],
                                     es, start=(tcc == 0),
                                     stop=(tcc == n_k_chunks - 1))
                den = sb.tile([65, TO], F32R, tag="den", bufs=2)
                nc.vector.tensor_copy(out=den[64:65, :], in_=av[64:65, :])
                rd = ps_pool.tile([64, TO], F32, tag="rd", bufs=2)
                nc.tensor.matmul(rd, ones64[64:65, :], den[64:65, :],
                                 start=True, stop=True)
                rd_sb = sb.tile([64, TO], F32, tag="rsb", bufs=2, name="rd_sb")
                nc.vector.reciprocal_approx_fast(out=rd_sb, in_=rd)
                if p0 == 0:
                    nc.vector.tensor_mul(out=o_all[0:64, ic, :], in0=av[0:64, :],
                                         in1=rd_sb)
                else:
                    o_tmp = sb.tile([64, TO], BF16, tag="ot", bufs=1)
                    nc.vector.tensor_mul(out=o_tmp, in0=av[0:64, :], in1=rd_sb)
                    nc.sync.dma_start(out=o_all[p0:p0 + 64, ic, :], in_=o_tmp)

        def wo_phase(ps_pool, w_dram, o_all, d_t, u_dram, bias_key, x_res, x2_dst):
            """x2 = x_res + W_o.T@O + u_o.T@(d_o.T@O) + b_o   (bf16 weights)."""
            od_ps = ps_pool.tile([RANK, TO], F32, tag="pj", bufs=4)
            for c in range(NCH):
                nc.tensor.matmul(od_ps, d_t[:, c, :], o_all[:, c, :],
                                 start=(c == 0), stop=(c == NCH - 1))
            od_sb = sb.tile([RANK, TO], BF16, tag="odx", bufs=2)
            nc.vector.tensor_copy(out=od_sb, in_=od_ps)
            for h in range(2):
                pss = [ps_pool.tile([128, TO], F32, tag="pj", bufs=4,
                                    name=f"pjo{h}_{i}") for i in range(4)]
                for c in range(NCH):
                    wt = sb.tile([128, 512], BF16, tag="wst", bufs=3)
                    nc.scalar.dma_start(out=wt,
                                        in_=chunked(w_dram)[:, c, h * 512:(h + 1) * 512])
                    for i in range(4):
                        nc.tensor.matmul(pss[i], wt[:, i * 128:(i + 1) * 128],
                                         o_all[:, c, :], start=(c == 0), stop=False)
                for i in range(4):
                    dc = 4 * h + i
                    ut = u_slice(u_dram, dc * 128, (dc + 1) * 128, BF16)
                    nc.tensor.matmul(pss[i], ut, od_sb, start=False, stop=True)
                    t = sb.tile([128, TO], F32, tag="cp", bufs=3)
                    nc.vector.tensor_scalar_add(out=t, in0=pss[i],
                                                scalar1=lnp[bias_key][:, dc:dc + 1])
                    nc.vector.tensor_add(out=x2_dst[:, dc, :], in0=t,
                                         in1=x_res(dc))

        # ======================= phase A: LN1 + Q/K/V =======================

        with tc.tile_pool(name="psA", bufs=1, space="PSUM") as psA:
            for tt in range(NTT):
                x_tt = sb.tile([128, NCH, TO], F32R, tag="xs", bufs=1)
                nc.sync.dma_start(out=x_tt,
                                  in_=chunked(xT)[:, :, tt * TO:(tt + 1) * TO])
                xs = [x_tt[:, c, :] for c in range(NCH)]
                m_sb, rstd = ln_stats(psA, xs, NCH)
                h1 = sb.tile([128, NCH, TO], BF16, tag="h1", bufs=1)
                h1c = [h1[:, c, :] for c in range(NCH)]
                ln_apply(psA, xs, h1c, m_sb, rstd, "ln1g", "ln1b", NCH)

                # packed qkv lora down: [96, TO]
                xd_ps = psA.tile([96, TO], F32, tag="pj", bufs=4)
                for c in range(NCH):
                    nc.tensor.matmul(xd_ps, d_qkv_t[:, c, :], h1c[c],
                                     start=(c == 0), stop=(c == NCH - 1))
                xd_tt = sb.tile([96, TO], BF16, tag="xd", bufs=2)
                nc.vector.tensor_copy(out=xd_tt, in_=xd_ps)

                if tt == 0:
                    def wq_out(ic, ps):
                        nc.vector.tensor_copy(out=Q_all[:, ic, :], in_=ps)
                    proj_T(psA, w_q, h1c, wq_out,
                           lora=(u_q, xd_tt[0:16, :], BF16))

                def wk_out(ic, ps, _tt=tt):
                    nc.vector.tensor_copy(
                        out=K_all[:, ic, _tt * TO:(_tt + 1) * TO], in_=ps)
                proj_T(psA, w_k, h1c, wk_out,
                       lora=(u_k, xd_tt[32:48, :], BF16), lrow=32)

                # V natural layout, 65-padded heads
                for half in range(2):
                    ps_v = [psA.tile([128, TO], F32, tag="pj", bufs=4,
                                     name=f"psv{half}_{i}") for i in range(4)]
                    for c in range(NCH):
                        wt = sb.tile([128, 512], BF16, tag="wst", bufs=3)
                        nc.scalar.dma_start(
                            out=wt,
                            in_=chunked(w_v)[:, c, half * 512:(half + 1) * 512])
                        for tcc in range(4):
                            nc.tensor.matmul(
                                ps_v[tcc], h1[:, c, tcc * 128:(tcc + 1) * 128],
                                wt, start=(c == 0), stop=False)
                    ut = u_slice(u_v, half * 512, (half + 1) * 512, row0=64)
                    for tcc in range(4):
                        nc.tensor.matmul(
                            ps_v[tcc],
                            xd_tt[64:80, tcc * 128:(tcc + 1) * 128],
                            ut, start=False, stop=True)
                        vtile = V_all[:, tt * 4 + tcc, :] \
                            .rearrange("p (h e) -> p h e", e=65)
                        nc.vector.tensor_copy(
                            out=vtile[:, half * 8:(half + 1) * 8, 0:64],
                            in_=ps_v[tcc].rearrange("p (h e) -> p h e", e=64))

        # ======================= phase B: self-attention ====================
        with tc.tile_pool(name="psB", bufs=1, space="PSUM") as psB:
            attn(psB, Q_all, K_all, lambda tcc: V_all[:, tcc, :], TF // 128, TF,
                 O_all)

        # ======================= phase C: Wo + residual =====================
        x2_all = sb.tile([128, NCH, TO], F32R, tag="x2")
        with tc.tile_pool(name="psC", bufs=1, space="PSUM") as psC:
            xres = sb.tile([128, NCH, TO], F32R, tag="xs", bufs=1)
            nc.sync.dma_start(out=xres, in_=chunked(xT)[:, :, 0:TO])
            wo_phase(psC, w_o, O_all, d_o_t, u_o, "b_o",
                     lambda dc: xres[:, dc, :], x2_all)

        # ============== phase D: LN2, cross-attention =======================
        Q2_all = sb.tile([128, NCH, TO], BF16, tag="qx")
        K2_all = sb.tile([128, NCH, NCTXP], BF16, tag="k2")
        V2_t = sb.tile([128, 16 * 65], BF16, tag="v2")
        nc.sync.dma_start(
            out=V2_t.rearrange("p (h e) -> p h e", e=65)[:, :, 64:65],
            in_=constsb[:, 0:16].rearrange("p (a b) -> p a b", b=1))
        ctx_t = sb.tile([128, CCH, NCTXP], BF16, tag="ctx")
        nc.sync.dma_start(out=ctx_t, in_=chunked(ctxT))
        O2_all = sb.tile([128, NCH, TO], BF16, tag="ox")

        with tc.tile_pool(name="psD", bufs=1, space="PSUM") as psD:
            x2c = [x2_all[:, c, :] for c in range(NCH)]
            m_sb, rstd = ln_stats(psD, x2c, NCH)
            h2 = sb.tile([128, NCH, TO], BF16, tag="h1", bufs=1)
            h2c = [h2[:, c, :] for c in range(NCH)]
            ln_apply(psD, x2c, h2c, m_sb, rstd, "ln2g", "ln2b", NCH)

            q2d_ps = psD.tile([RANK, TO], F32, tag="pj", bufs=4)
            for c in range(NCH):
                nc.tensor.matmul(q2d_ps, d_q2_t[:, c, :], h2c[c],
                                 start=(c == 0), stop=(c == NCH - 1))
            q2d_sb = sb.tile([RANK, TO], BF16, tag="odx", bufs=2)
            nc.vector.tensor_copy(out=q2d_sb, in_=q2d_ps)

            def wq2_out(ic, ps):
                nc.vector.tensor_copy(out=Q2_all[:, ic, :], in_=ps)
            proj_T(psD, w_q2, h2c, wq2_out, lora=(u_q2, q2d_sb, BF16))

            # kv2 lora down from raw context
            cd_ps = psD.tile([48, NCTXP], F32, tag="pj", bufs=4)
            for c in range(CCH):
                nc.tensor.matmul(cd_ps, d_kv2_t[:, c, :], ctx_t[:, c, :],
                                 start=(c == 0), stop=(c == CCH - 1))
            cd_sb = sb.tile([48, NCTXP], BF16, tag="odx", bufs=2)
            nc.vector.tensor_copy(out=cd_sb, in_=cd_ps)

            # K2^T
            for h in range(2):
                pss = [psD.tile([128, NCTXP], F32, tag="pj", bufs=4,
                                name=f"pk2{h}_{i}") for i in range(4)]
                for c in range(CCH):
                    wt = sb.tile([128, 512], BF16, tag="wst", bufs=3)
                    nc.scalar.dma_start(
                        out=wt, in_=chunked(w_k2)[:, c, h * 512:(h + 1) * 512])
                    for i in range(4):
                        nc.tensor.matmul(pss[i], wt[:, i * 128:(i + 1) * 128],
                                         ctx_t[:, c, :], start=(c == 0), stop=False)
                for i in range(4):
                    ic = 4 * h + i
                    ut = u_slice(u_k2, ic * 128, (ic + 1) * 128)
                    nc.tensor.matmul(pss[i], ut, cd_sb[0:16, :], start=False,
                                     stop=True)
                    nc.vector.tensor_copy(out=K2_all[:, ic, :], in_=pss[i])
            # V2 natural
            for half in range(2):
                ps_v = psD.tile([NCTX, 512], F32, tag="pj", bufs=4)
                for c in range(CCH):
                    wt = sb.tile([128, 512], BF16, tag="wst", bufs=3)
                    nc.scalar.dma_start(
                        out=wt, in_=chunked(w_v2)[:, c, half * 512:(half + 1) * 512])
                    nc.tensor.matmul(ps_v, ctx_t[:, c, 0:NCTX], wt,
                                     start=(c == 0), stop=False)
                ut = u_slice(u_v2, half * 512, (half + 1) * 512, row0=32)
                nc.tensor.matmul(ps_v, cd_sb[32:48, 0:NCTX], ut, start=False, stop=True)
                nc.vector.tensor_copy(
                    out=V2_t.rearrange("p (h e) -> p h e", e=65)
                        [0:NCTX, half * 8:(half + 1) * 8, 0:64],
                    in_=ps_v.rearrange("p (h e) -> p h e", e=64))

        with tc.tile_pool(name="psD2", bufs=1, space="PSUM") as psD2:
            attn(psD2, Q2_all, K2_all, lambda tcc: V2_t, 1, NCTX, O2_all)

        # ======================= phase E: Wo2 + residual ====================
        x3_all = sb.tile([128, NCH, TO], F32R, tag="x3")
        with tc.tile_pool(name="psE", bufs=1, space="PSUM") as psE:
            wo_phase(psE, w_o2, O2_all, d_o2_t, u_o2, "b_o2",
                     lambda dc: x2_all[:, dc, :], x3_all)

        # ======================= phase F: LN3 ===============================
        h3 = sb.tile([128, NCH, TO], BF16, tag="h1", bufs=1)
        with tc.tile_pool(name="psF", bufs=1, space="PSUM") as psF:
            x3c = [x3_all[:, c, :] for c in range(NCH)]
            m_sb, rstd = ln_stats(psF, x3c, NCH)
            h3c = [h3[:, c, :] for c in range(NCH)]
            ln_apply(psF, x3c, h3c, m_sb, rstd, "ln3g", "ln3b", NCH)

        # ======================= phase G: GEGLU FF ==========================
        h_lo = sb.tile([128, 16, TO], BF16, tag="kx")   # h chunks 0..15
        h_hi = sb.tile([128, 16, TO], BF16, tag="vx")   # h chunks 16..31
        with tc.tile_pool(name="psG", bufs=1, space="PSUM") as psG:
            h3c = [h3[:, c, :] for c in range(NCH)]
            pd_ps = psG.tile([RANK, TO], F32, tag="pj", bufs=4)
            for c in range(NCH):
                nc.tensor.matmul(pd_ps, d_p_t[:, c, :], h3c[c],
                                 start=(c == 0), stop=(c == NCH - 1))
            pd_sb = sb.tile([RANK, TO], BF16, tag="odx", bufs=2)
            nc.vector.tensor_copy(out=pd_sb, in_=pd_ps)

            def proj_block(j, gate):
                """proj^T chunks for col block j (4 chunks of 128).
                Returns [(fc, psum_tile)]."""
                base = (32 if gate else 0) + 4 * j
                pss = [psG.tile([128, TO], F32, tag="pj", bufs=4,
                                name=f"pg{h}_{i}") for i in range(4)]
                for c in range(NCH):
                    wt = sb.tile([128, 512], BF16, tag="wst", bufs=3)
                    nc.scalar.dma_start(
                        out=wt,
                        in_=chunked(w_p)[:, c, base * 128:(base + 4) * 128])
                    for i in range(4):
                        nc.tensor.matmul(pss[i], wt[:, i * 128:(i + 1) * 128],
                                         h3c[c], start=(c == 0), stop=False)
                out = []
                for i in range(4):
                    fc = base + i
                    ut = u_slice(u_p, fc * 128, (fc + 1) * 128)
                    nc.tensor.matmul(pss[i], ut, pd_sb, start=False, stop=True)
                    out.append((fc, pss[i]))
                return out

            def h_ap(fc):
                return h_lo[:, fc, :] if fc < 16 else h_hi[:, fc - 16, :]

            for j in range(8):
                gels = []
                for fc, ps in proj_block(j, gate=True):
                    gel = sb.tile([128, TO], F32, tag="gel", bufs=4)
                    nc.scalar.activation(out=gel, in_=ps, func=AF.Gelu,
                                         bias=bp_t[:, fc:fc + 1])
                    gels.append(gel)
                for idx, (fc, ps) in enumerate(proj_block(j, gate=False)):
                    t = sb.tile([128, TO], F32, tag="cp", bufs=3)
                    nc.vector.tensor_scalar_add(out=t, in0=ps,
                                                scalar1=bp_t[:, fc:fc + 1])
                    nc.vector.tensor_mul(out=h_ap(fc), in0=t, in1=gels[idx])

            # second FF matmul + bias + residual -> outT
            hd_ps = psG.tile([RANK, TO], F32, tag="pj", bufs=4)
            for fc in range(32):
                nc.tensor.matmul(hd_ps, d_f_t[:, fc, :], h_ap(fc),
                                 start=(fc == 0), stop=(fc == 31))
            hd_sb = sb.tile([RANK, TO], BF16, tag="odx", bufs=2)
            nc.vector.tensor_copy(out=hd_sb, in_=hd_ps)
            for h in range(2):
                pss = [psG.tile([128, TO], F32, tag="pj", bufs=4,
                                name=f"pg{h}_{i}") for i in range(4)]
                for fc in range(32):
                    wt = sb.tile([128, 512], BF16, tag="wst", bufs=3)
                    nc.scalar.dma_start(
                        out=wt, in_=chunked(w_2)[:, fc, h * 512:(h + 1) * 512])
                    for i in range(4):
                        nc.tensor.matmul(pss[i], wt[:, i * 128:(i + 1) * 128],
                                         h_ap(fc), start=(fc == 0), stop=False)
                for i in range(4):
                    dc = 4 * h + i
                    ut = u_slice(u_f, dc * 128, (dc + 1) * 128)
                    nc.tensor.matmul(pss[i], ut, hd_sb, start=False, stop=True)
                    t = sb.tile([128, TO], F32, tag="cp", bufs=3)
                    nc.vector.tensor_scalar_add(out=t, in0=pss[i],
                                                scalar1=lnp["b_2"][:, dc:dc + 1])
                    of = sb.tile([128, TO], F32, tag="gel", bufs=4)
                    nc.vector.tensor_add(out=of, in0=t, in1=x3_all[:, dc, :])
                    nc.sync.dma_start(out=outT[dc * 128:(dc + 1) * 128, :], in_=of)

    nc.finalize()
    return nc


_CACHE = {}


def _get_nc():
    if "nc" not in _CACHE:
        _CACHE["nc"] = _build(bacc.Bacc())
    return _CACHE["nc"]


def _prep_in_maps(x, context, params):
    p = params
    sc = lambda a: float(a) * 1.0 / RANK  # LORA_W * alpha / rank

    def f32(a):
        return np.ascontiguousarray(np.asarray(a, dtype=np.float32))

    def bf(a):
        return np.ascontiguousarray(np.asarray(a, dtype=np.float32)
                                    .astype(BF16NP))

    a1, a2, ff = p["attn1"], p["attn2"], p["ff"]
    shared = {
        "consts": np.concatenate(
            [np.ones((128, 128), np.float32),
             np.full((128, 1), 1.0 / DIM, np.float32)], 1),
        "constsb": np.ones((128, 256), np.float32).astype(BF16NP),
        "w_q": bf(a1["Wq"]), "w_k": bf(a1["Wk"]), "w_v": bf(a1["Wv"]),
        "w_o": bf(a1["Wo"]), "b_o": f32(a1["bo"]),
        "u_q": bf(np.asarray(a1["qu"]) * sc(a1["qa"])),
        "u_k": bf(np.asarray(a1["ku"]) * sc(a1["ka"])),
        "u_v": bf(np.asarray(a1["vu"]) * sc(a1["va"])),
        "d_o": bf(a1["od"]), "u_o": bf(np.asarray(a1["ou"]) * sc(a1["oa"])),
        "w_q2": bf(a2["Wq"]), "w_k2": bf(a2["Wk"]), "w_v2": bf(a2["Wv"]),
        "w_o2": bf(a2["Wo"]), "b_o2": f32(a2["bo"]),
        "d_q2": bf(a2["qd"]), "u_q2": bf(np.asarray(a2["qu"]) * sc(a2["qa"])),
        "u_k2": bf(np.asarray(a2["ku"]) * sc(a2["ka"])),
        "u_v2": bf(np.asarray(a2["vu"]) * sc(a2["va"])),
        "d_o2": bf(a2["od"]), "u_o2": bf(np.asarray(a2["ou"]) * sc(a2["oa"])),
        "ln1g": f32(p["ln1_g"]), "ln1b": f32(p["ln1_b"]),
        "ln2g": f32(p["ln2_g"]), "ln2b": f32(p["ln2_b"]),
        "ln3g": f32(p["ln3_g"]), "ln3b": f32(p["ln3_b"]),
        "w_p": bf(ff["Wp"]), "b_p": f32(ff["bp"]),
        "d_p": bf(ff["pd"]), "u_p": bf(np.asarray(ff["pu"]) * sc(ff["pa"])),
        "w_2": bf(ff["W2"]), "b_2": f32(ff["b2"]),
        "d_f": bf(ff["fd"]), "u_f": bf(np.asarray(ff["fu"]) * sc(ff["fa"])),
    }
    d_qkv = np.zeros((DIM, 96), BF16NP)
    d_qkv[:, 0:16] = np.asarray(a1["qd"]); d_qkv[:, 32:48] = np.asarray(a1["kd"])
    d_qkv[:, 64:80] = np.asarray(a1["vd"])
    shared["d_qkv"] = d_qkv
    d_kv2 = np.zeros((CTX_DIM, 48), BF16NP)
    d_kv2[:, 0:16] = np.asarray(a2["kd"]); d_kv2[:, 32:48] = np.asarray(a2["vd"])
    shared["d_kv2"] = d_kv2

    x = np.asarray(x, np.float32)
    context = np.asarray(context, np.float32)
    in_maps = []
    for core in range(N_CORES):
        b, t0 = core // 4, (core % 4) * TO
        xt = x[b].T  # [DIM, TF]
        m = dict(shared)
        m["xT"] = np.ascontiguousarray(
            np.concatenate([xt[:, t0:], xt[:, :t0]], axis=1))
        ctp = np.zeros((CTX_DIM, NCTXP), BF16NP)
        ctp[:, :NCTX] = context[b].T.astype(BF16NP)
        m["ctxT"] = ctp
        in_maps.append(m)
    return in_maps


def run_spmd(in_maps, **kw):
    return run_bass_kernel_spmd(_get_nc(), in_maps,
                                core_ids=list(range(N_CORES)), **kw)


def kernel(x, context, params):
    in_maps = _prep_in_maps(x, context, params)
    res = run_spmd(in_maps)
    B, N = np.asarray(x).shape[:2]
    out = np.empty((B, N, DIM), np.float32)
    for core in range(N_CORES):
        b, t0 = core // 4, (core % 4) * TO
        out[b, t0:t0 + TO, :] = res.results[core]["outT"].T
    return out
